# revision 1
# baseline (speedup 1.0000x reference)
"""LDPC belief-propagation kernel for Trainium2 (8 NeuronCores, data-parallel).

Math (per batch row, H fixed [3,7], 12 edges, check-major edge order):
  lu_e  = ln|tanh(m_e/2)|           = ln(1-z) - ln(1+z),  z = exp(-|m_e|)
  S_c   = sum_{e in check c} lu_e
  d_e   = S_c - lu_e                (== s_upd, <= 0)
  mag_e = -ln tanh(|d_e|/2)         = ln(1+u) - ln(1-u),  u = exp(d_e)
  sgn_e = prod_{e' in c} sign(m_{e'}) * sign(m_e)    (leave-one-out, +-1)
  c2v_e = mag_e * sgn_e
  new_llr_v = llr_v + sum_{c contains v} c2v_{c,v}
  m'_e  = new_llr_v - c2v_e
Only Exp/Ln/Abs/Sign activations -> one ACT table set, no table switches.
Edges of degree-1 variables (e0,e4,e8) carry constant messages == llr: their
lu/sign are computed once; per-iteration transcendentals cover only the 9
dynamic edges, and deg-1 new_llr terms are added only on the last iteration.
Batch is split into chunks so ACT/DVE/GPSIMD/DMA pipeline across chunks.
"""

import numpy as np

_CACHE = {}

NCORES = 8
P = 128      # partitions
CHUNKS = 2   # batch sub-chunks per core (pipeline depth)


def _build(Bc, iters):
    import contextlib

    import concourse.bass as bass
    import concourse.tile as tile
    from concourse import mybir
    from concourse.alu_op_type import AluOpType as Op

    F = mybir.ActivationFunctionType
    W = Bc // P // CHUNKS  # free columns per partition per chunk
    f32 = mybir.dt.float32

    nc = bass.Bass("TRN2", target_bir_lowering=False, debug=False,
                   num_devices=1)
    llr_d = nc.dram_tensor("llr", [Bc, 7], f32, kind="ExternalInput")
    out_d = nc.dram_tensor("out", [Bc, 7], f32, kind="ExternalOutput")

    def sub(t, off, dims):
        a = t[:] if callable(getattr(t, "__getitem__", None)) else t
        return bass.AP(tensor=a.tensor, offset=a.offset + off,
                       ap=[list(a.ap[0])] + [list(d) for d in dims])

    with tile.TileContext(nc) as tc:
        ctx = contextlib.ExitStack()
        with ctx:
            keep = ctx.enter_context(tc.tile_pool(name="keep", bufs=1))
            work = ctx.enter_context(tc.tile_pool(name="work", bufs=2))

            def K(name, k):
                return keep.tile([P, W * k], f32, tag=name, name=name)

            CB = keep.tile([P, 1], f32, tag="CB", name="CB")
            nc.vector.memset(CB, 1e-38)
            CB2 = keep.tile([P, 1], f32, tag="CB2", name="CB2")
            nc.vector.memset(CB2, 0.99999994)

            # per-chunk persistent state
            LLRs = [K(f"LLR{c}", 7) for c in range(CHUNKS)]
            Ms   = [K(f"M{c}", 12) for c in range(CHUNKS)]
            LUs  = [K(f"LU{c}", 12) for c in range(CHUNKS)]
            SGs  = [K(f"SG{c}", 12) for c in range(CHUNKS)]
            NLs  = [K(f"NL{c}", 7) for c in range(CHUNKS)]

            act = nc.scalar.activation
            vec = nc.vector
            gps = nc.gpsimd

            def g12(t):
                return sub(t, 0, [[12, W], [4, 3], [1, 4]])

            def dyn9(t):
                return sub(t, 1, [[12, W], [4, 3], [1, 3]])

            llr_ap = llr_d.ap().rearrange("(c p w) v -> c p (w v)", c=CHUNKS, p=P)
            out_ap = out_d.ap().rearrange("(c p w) v -> c p (w v)", c=CHUNKS, p=P)

            for c in range(CHUNKS):
                LLR, M = LLRs[c], Ms[c]
                nc.sync.dma_start(out=LLR[:], in_=llr_ap[c])
                vec.tensor_copy(sub(M, 0, [[12, W], [1, 4]]),
                                sub(LLR, 0, [[7, W], [2, 4]]))
                vec.tensor_copy(sub(M, 4, [[12, W], [1, 2]]),
                                sub(LLR, 1, [[7, W], [1, 2]]))
                vec.tensor_copy(sub(M, 6, [[12, W], [1, 2]]),
                                sub(LLR, 5, [[7, W], [1, 2]]))
                vec.tensor_copy(sub(M, 8, [[12, W], [1, 4]]),
                                sub(LLR, 3, [[7, W], [1, 4]]))

            for it in range(iters):
                full = (it == 0)
                lastit = (it == iters - 1)
                for c in range(CHUNKS):
                    LLR, M, LU, SG, NL = LLRs[c], Ms[c], LUs[c], SGs[c], NLs[c]
                    # scratch (tag-shared slots rotate across chunk bodies)
                    ZU  = work.tile([P, W * 12], f32, tag="ZU", name="ZU")
                    LPR = work.tile([P, W * 12], f32, tag="LPR", name="LPR")
                    LQS = work.tile([P, W * 12], f32, tag="LQS", name="LQS")
                    T6  = work.tile([P, W * 6], f32, tag="T6", name="T6")
                    S3  = work.tile([P, W * 3], f32, tag="S3", name="S3")
                    G6  = work.tile([P, W * 6], f32, tag="G6", name="G6")
                    G3  = work.tile([P, W * 3], f32, tag="G3", name="G3")
                    DM  = work.tile([P, W * 12], f32, tag="DM", name="DM")
                    SL  = work.tile([P, W * 12], f32, tag="SL", name="SL")
                    CV  = work.tile([P, W * 12], f32, tag="CV", name="CV")
                    TP  = work.tile([P, W * 2], f32, tag="TP", name="TP")

                    sl = (lambda t: t[:]) if full else dyn9
                    # phi1: lu = ln(1-z) - ln(1+z), z = exp(-|m|) clamped < 1
                    act(sl(ZU), sl(M), F.Abs)
                    act(sl(ZU), sl(ZU), F.Exp, scale=-1.0)
                    act(sl(LPR), sl(ZU), F.Ln, bias=1.0)
                    # scale/bias chosen so the argument stays >= 6e-8 even at
                    # z == 1.0 (m == +-0): keeps lu finite and strictly < 0
                    act(sl(LQS), sl(ZU), F.Ln, bias=CB2[:], scale=-0.99999988)
                    vec.tensor_tensor(sl(LU), sl(LQS), sl(LPR), Op.subtract)
                    # sign (+1 at exact zero via tiny bias)
                    act(sl(SG), sl(M), F.Sign, bias=CB[:])

                    # check sums / sign products
                    vec.tensor_tensor(T6[:], sub(LU, 0, [[12, W], [4, 3], [1, 2]]),
                                      sub(LU, 2, [[12, W], [4, 3], [1, 2]]), Op.add)
                    vec.tensor_tensor(S3[:], sub(T6, 0, [[6, W], [2, 3]]),
                                      sub(T6, 1, [[6, W], [2, 3]]), Op.add)
                    gps.tensor_tensor(G6[:], sub(SG, 0, [[12, W], [4, 3], [1, 2]]),
                                      sub(SG, 2, [[12, W], [4, 3], [1, 2]]), Op.mult)
                    gps.tensor_tensor(G3[:], sub(G6, 0, [[6, W], [2, 3]]),
                                      sub(G6, 1, [[6, W], [2, 3]]), Op.mult)

                    slg = g12 if lastit else dyn9
                    slf = (lambda t: t[:]) if lastit else dyn9
                    S3r = sub(S3, 0, [[3, W], [1, 3], [0, 4 if lastit else 3]])
                    G3r = sub(G3, 0, [[3, W], [1, 3], [0, 4 if lastit else 3]])
                    vec.tensor_tensor(slg(DM), S3r, slg(LU), Op.subtract)
                    act(slf(ZU), slf(DM), F.Exp)
                    act(slf(LPR), slf(ZU), F.Ln, bias=1.0)
                    act(slf(LQS), slf(ZU), F.Ln, bias=1.0, scale=-1.0)
                    gps.tensor_tensor(slg(SL), G3r, slg(SG), Op.mult)
                    vec.tensor_tensor(slf(DM), slf(LPR), slf(LQS), Op.subtract)
                    vec.tensor_tensor(slf(CV), slf(DM), slf(SL), Op.mult)

                    # new_llr for feedback vars v2,v5 (pairs), v4, v6
                    vec.tensor_tensor(TP[:], sub(CV, 1, [[12, W], [5, 2]]),
                                      sub(CV, 5, [[12, W], [5, 2]]), Op.add)
                    vec.tensor_tensor(sub(NL, 2, [[7, W], [3, 2]]),
                                      sub(LLR, 2, [[7, W], [3, 2]]),
                                      TP[:], Op.add)
                    vec.tensor_tensor(sub(NL, 4, [[7, W], [2, 2]]),
                                      sub(LLR, 4, [[7, W], [2, 2]]),
                                      sub(CV, 2, [[12, W], [1, 2]]), Op.add)
                    vec.tensor_tensor(sub(NL, 4, [[7, W], [2, 2]]),
                                      sub(NL, 4, [[7, W], [2, 2]]),
                                      sub(CV, 9, [[12, W], [-2, 2]]), Op.add)
                    vec.tensor_tensor(sub(NL, 6, [[7, W], [1, 1]]),
                                      sub(NL, 6, [[7, W], [1, 1]]),
                                      sub(CV, 11, [[12, W], [1, 1]]), Op.add)

                    if lastit:
                        vec.tensor_tensor(sub(NL, 0, [[7, W], [1, 2]]),
                                          sub(LLR, 0, [[7, W], [1, 2]]),
                                          sub(CV, 0, [[12, W], [4, 2]]), Op.add)
                        vec.tensor_tensor(sub(NL, 3, [[7, W], [1, 1]]),
                                          sub(LLR, 3, [[7, W], [1, 1]]),
                                          sub(CV, 8, [[12, W], [1, 1]]), Op.add)
                        nc.sync.dma_start(out=out_ap[c], in_=NL[:])
                    else:
                        # m' = new_llr - c2v for the 9 dynamic edges
                        vec.tensor_tensor(sub(M, 1, [[12, W], [1, 3]]),
                                          sub(NL, 2, [[7, W], [2, 3]]),
                                          sub(CV, 1, [[12, W], [1, 3]]), Op.subtract)
                        vec.tensor_tensor(sub(M, 9, [[12, W], [1, 3]]),
                                          sub(NL, 4, [[7, W], [1, 3]]),
                                          sub(CV, 9, [[12, W], [1, 3]]), Op.subtract)
                        vec.tensor_tensor(sub(M, 5, [[12, W], [1, 1]]),
                                          sub(NL, 2, [[7, W], [1, 1]]),
                                          sub(CV, 5, [[12, W], [1, 1]]), Op.subtract)
                        vec.tensor_tensor(sub(M, 6, [[12, W], [1, 2]]),
                                          sub(NL, 5, [[7, W], [1, 2]]),
                                          sub(CV, 6, [[12, W], [1, 2]]), Op.subtract)

    # walrus on this stack supports a single sync-wait slot per instruction.
    # Tile emits (a) redundant same-engine waits (trivially satisfied by the
    # engine's FIFO program order once the preceding updates have happened)
    # and (b) a kernel-tail SP drain waiting on the whole global clock, where
    # only the output-DMA wait is load-bearing (the per-engine drain + EVSEM
    # butterfly that follows enforces engine completion).  Strip both.
    import bass_rust
    pref = {"EngineType.DVE": "DVE_", "EngineType.Pool": "Pool_",
            "EngineType.Activation": "Activation_", "EngineType.PE": "PE_",
            "EngineType.SP": "SP_"}
    inc = {}
    for b in nc.m.functions[0].blocks:
        for i in b.instructions:
            si = i.sync_info
            if si is None:
                continue
            if len(si.on_wait) > 1:
                if type(i).__name__ == "InstDrain":
                    dma = [w for w in si.on_wait if "DMA" in w.ant_name]
                    keep_w = dma[-1:] if dma else list(si.on_wait)[:1]
                else:
                    p = pref.get(str(i.engine))
                    keep_w = [w for w in si.on_wait
                              if not (p and w.ant_name.startswith(p)
                                      and w.wait_value <= inc.get(w.ant_name, 0))]
                    assert len(keep_w) <= 1, (i.name, [(w.ant_name, w.wait_value) for w in keep_w], {k: inc.get(k) for k in [w.ant_name for w in si.on_wait]})
                i.sync_info = bass_rust.SyncInfo(on_wait=keep_w,
                                                on_update=list(si.on_update))
                si = i.sync_info
            for u in si.on_update:
                if u.update_mode == "sem-inc":
                    inc[u.ant_name] = inc.get(u.ant_name, 0) + u.update_value
    return nc


def kernel(llr, max_iters):
    llr = np.ascontiguousarray(np.asarray(llr), dtype=np.float32)
    iters = int(np.asarray(max_iters))
    B = llr.shape[0]
    if iters <= 0:
        return llr.reshape(B, 1, 7).copy()

    from concourse.bass_utils import run_bass_kernel_spmd

    Bc = B // NCORES
    key = (Bc, iters)
    if key not in _CACHE:
        _CACHE[key] = _build(Bc, iters)
    nc = _CACHE[key]

    flat = llr.reshape(B, 7)
    in_maps = [{"llr": flat[i * Bc:(i + 1) * Bc]} for i in range(NCORES)]
    res = run_bass_kernel_spmd(nc, in_maps, core_ids=list(range(NCORES)))
    out = np.concatenate([np.asarray(r["out"]) for r in res.results], axis=0)
    return out.reshape(B, 1, 7)



# revision 10
# speedup vs baseline: 2.1774x; 2.1774x over previous
"""LDPC belief-propagation kernel for Trainium2 (8 NeuronCores, data-parallel).

Math (per batch row, H fixed [3,7], 12 edges):
  t_e   = tanh(m_e / 2)                       (signed!)
  u_e   = prod_{e' in check c, e' != e} t_e'  (signed leave-one-out product)
  c2v_e = 2 atanh(u_e) = ln(1+u) - ln(1-u)    (signed, sign handled for free)
  new_llr_v = llr_v + sum_{c contains v} c2v_{c,v}
  m'_e  = new_llr_v - c2v_e
Only Tanh/Ln tables; the sign pipeline of the classic phi/phi formulation
disappears because the tanh products carry signs natively.

Edge layout is role-major per 12-slot group: [s0 s1 s2 | a0 a1 a2 | b0 b1 b2 |
d0 d1 d2] where s_c is the check's degree-1-variable edge (v0,v1,v3 - their
messages never change), d_c is v6's edge in check c, and (a, b) =
((v2c0, v2c1, v4c2), (v4c0, v5c1, v5c2)).  This makes every structural op a
single strided instruction:
  Q[k]   = T[k] * T[k+6]         (k=0..5: pair products (s*b, a*d) per check)
  U[3..8]  = T[(9,10,11,0,1,2)] * Q[0..5]   (loo for roles a, b)
  U[9..11] = T[3..5] * Q[0..2]              (loo for role d)
  M'[deg2 six edges] = LB6 + CV[partner]    (partner = pair-swap view)
  M'[d]  = (c2v_d-sum + llr_6) - CV[d]      (v6 leave-one-out via total sum)
Batch is split into 2 chunks so ACT (Tanh/Ln/Ln), DVE (products, M-updates)
and Pool (CV subtract, v6 sums) pipeline across chunks.
"""

import numpy as np

_CACHE = {}

NCORES = 8
P = 128      # partitions
CHUNKS = 2   # batch sub-chunks per core (pipeline depth)

# guard so ln(1 -+ 0.99999988*u) stays finite (>= ~1.2e-7) even at u = -+1
LNSCALE = 0.99999988


def _build(Bc, iters):
    import contextlib

    import concourse.bass as bass
    import concourse.tile as tile
    from concourse import mybir
    from concourse.alu_op_type import AluOpType as Op

    F = mybir.ActivationFunctionType
    W = Bc // P // CHUNKS  # free columns per partition per chunk
    f32 = mybir.dt.float32

    nc = bass.Bass("TRN2", target_bir_lowering=False, debug=False,
                   num_devices=1)
    llr_d = nc.dram_tensor("llr", [Bc, 7], f32, kind="ExternalInput")
    out_d = nc.dram_tensor("out", [Bc, 7], f32, kind="ExternalOutput")

    def sub(t, off, dims):
        a = t[:] if callable(getattr(t, "__getitem__", None)) else t
        return bass.AP(tensor=a.tensor, offset=a.offset + off,
                       ap=[list(a.ap[0])] + [list(d) for d in dims])

    with tile.TileContext(nc) as tc:
        ctx = contextlib.ExitStack()
        with ctx:
            keep = ctx.enter_context(tc.tile_pool(name="keep", bufs=1))
            work = ctx.enter_context(tc.tile_pool(name="work", bufs=2))

            def K(name, k):
                return keep.tile([P, W * k], f32, tag=name, name=name)

            # per-chunk persistent state
            LLs = [K(f"LL{c}", 7) for c in range(CHUNKS)]    # llr, natural v order
            LBs = [K(f"LB{c}", 6) for c in range(CHUNKS)]    # llr bcast for deg2 edges
            Ts  = [K(f"T{c}", 12) for c in range(CHUNKS)]    # tanh(m/2) per edge
            Ms  = [K(f"M{c}", 9) for c in range(CHUNKS)]     # dyn messages (T slots 3..11)
            NLs = [K(f"NL{c}", 7) for c in range(CHUNKS)]    # output llr

            act = nc.scalar.activation
            vec = nc.vector
            gps = nc.gpsimd

            llr_ap = llr_d.ap().rearrange("(c p w) v -> c p (w v)", c=CHUNKS, p=P)
            out_ap = out_d.ap().rearrange("(c p w) v -> c p (w v)", c=CHUNKS, p=P)

            for c in range(CHUNKS):
                nc.sync.dma_start(out=LLs[c][:], in_=llr_ap[c])

            def v7(t, off, *dims):
                return sub(t, off, [[7, W]] + [list(d) for d in dims])

            def v12(t, off, *dims):
                return sub(t, off, [[12, W]] + [list(d) for d in dims])

            def v9(t, off, *dims):
                return sub(t, off, [[9, W]] + [list(d) for d in dims])

            def v6(t, off, *dims):
                return sub(t, off, [[6, W]] + [list(d) for d in dims])

            def v3(t, off, *dims):
                return sub(t, off, [[3, W]] + [list(d) for d in dims])

            def v1(t, off, *dims):
                return sub(t, off, [[1, W]] + [list(d) for d in dims])

            for it in range(iters):
                full = (it == 0)
                last = (it == iters - 1)
                for c in range(CHUNKS):
                    LL, LB, T, M, NL = LLs[c], LBs[c], Ts[c], Ms[c], NLs[c]
                    Q  = work.tile([P, W * 6], f32, tag="Q", name="Q")
                    U  = work.tile([P, W * 12], f32, tag="U", name="U")
                    LP = work.tile([P, W * 12], f32, tag="LP", name="LP")
                    LM = work.tile([P, W * 12], f32, tag="LM", name="LM")
                    CV = work.tile([P, W * 12], f32, tag="CV", name="CV")

                    if full:
                        # t = tanh(llr/2) once, scatter to role-major edge slots
                        TL = work.tile([P, W * 7], f32, tag="TL", name="TL")
                        act(TL[:], LL[:], F.Tanh, scale=0.5)
                        vec.tensor_copy(v12(T, 0, [1, 2]), v7(TL, 0, [1, 2]))
                        vec.tensor_copy(v12(T, 2, [1, 1]), v7(TL, 3, [1, 1]))
                        vec.tensor_copy(v12(T, 3, [1, 2]), v7(TL, 2, [0, 2]))
                        vec.tensor_copy(v12(T, 5, [1, 2]), v7(TL, 4, [0, 2]))
                        vec.tensor_copy(v12(T, 7, [1, 2]), v7(TL, 5, [0, 2]))
                        vec.tensor_copy(v12(T, 9, [1, 3]), v7(TL, 6, [0, 3]))
                        # llr broadcast for the six deg-2 edges (a|b role order)
                        vec.tensor_copy(v6(LB, 0, [1, 2]), v7(LL, 2, [0, 2]))
                        vec.tensor_copy(v6(LB, 2, [1, 4]), v7(LL, 4, [1, 2], [0, 2]))
                    else:
                        act(v12(T, 3, [1, 9]), M[:], F.Tanh, scale=0.5)

                    # pair products and signed leave-one-out products
                    vec.tensor_tensor(Q[:], v12(T, 0, [1, 6]), v12(T, 6, [1, 6]),
                                      Op.mult)
                    vec.tensor_tensor(v12(U, 3, [1, 6]),
                                      v12(T, 9, [-9, 2], [1, 3]),
                                      v6(Q, 0, [1, 6]), Op.mult)
                    gps.tensor_tensor(v12(U, 9, [1, 3]), v12(T, 3, [1, 3]),
                                      v6(Q, 0, [1, 3]), Op.mult)
                    if last:
                        gps.tensor_tensor(v12(U, 0, [1, 3]), v12(T, 6, [1, 3]),
                                          v6(Q, 3, [1, 3]), Op.mult)

                    off, n = (0, 9) if last else (3, 6)
                    # c2v = ln(1+u) - ln(1-u), guarded away from ln(0)
                    act(v12(LP, off, [1, n + 3]), v12(U, off, [1, n + 3]), F.Ln,
                        bias=1.0, scale=LNSCALE)
                    act(v12(LM, off, [1, n + 3]), v12(U, off, [1, n + 3]), F.Ln,
                        bias=1.0, scale=-LNSCALE)
                    vec.tensor_tensor(v12(CV, off, [1, n]), v12(LP, off, [1, n]),
                                      v12(LM, off, [1, n]), Op.subtract)
                    gps.tensor_tensor(v12(CV, 9, [1, 3]), v12(LP, 9, [1, 3]),
                                      v12(LM, 9, [1, 3]), Op.subtract)

                    # v6 leave-one-out sums of the d-role c2vs, depth 2:
                    # X[0]=c10+c11  X[1]=c9+c11  X[2]=c9+c10
                    X = work.tile([P, W * 3], f32, tag="X", name="X")
                    gps.tensor_tensor(v3(X, 0, [1, 2]), v12(CV, 10, [-1, 2]),
                                      v12(CV, 11, [0, 2]), Op.add)
                    gps.tensor_tensor(v3(X, 2, [1, 1]), v12(CV, 9, [1, 1]),
                                      v12(CV, 10, [1, 1]), Op.add)

                    if not last:
                        # m' for the six deg-2 edges: llr + partner c2v
                        vec.tensor_tensor(v9(M, 0, [1, 6]), v6(LB, 0, [1, 6]),
                                          v12(CV, 4, [2, 3], [-1, 2]), Op.add)
                        # m' for v6 edges: llr6 + sum of the other two c2v_d
                        gps.tensor_tensor(v9(M, 6, [1, 3]), v7(LL, 6, [0, 3]),
                                          v3(X, 0, [1, 3]), Op.add)
                    else:
                        # new_llr in natural variable order
                        SP = work.tile([P, W * 3], f32, tag="SP", name="SP")
                        vec.tensor_tensor(v7(NL, 0, [1, 2]), v7(LL, 0, [1, 2]),
                                          v12(CV, 0, [1, 2]), Op.add)
                        vec.tensor_tensor(v7(NL, 3, [1, 1]), v7(LL, 3, [1, 1]),
                                          v12(CV, 2, [1, 1]), Op.add)
                        vec.tensor_tensor(SP[:], v12(CV, 3, [2, 3]),
                                          v12(CV, 4, [2, 3]), Op.add)
                        vec.tensor_tensor(v7(NL, 2, [1, 1]), v7(LL, 2, [1, 1]),
                                          v3(SP, 0, [1, 1]), Op.add)
                        vec.tensor_tensor(v7(NL, 4, [1, 2]), v7(LL, 4, [1, 2]),
                                          v3(SP, 1, [1, 2]), Op.add)
                        S1 = work.tile([P, W], f32, tag="S1", name="S1")
                        gps.tensor_tensor(S1[:], v3(X, 2, [1, 1]),
                                          v12(CV, 11, [1, 1]), Op.add)
                        gps.tensor_tensor(v7(NL, 6, [1, 1]), S1[:],
                                          v7(LL, 6, [1, 1]), Op.add)
                        nc.sync.dma_start(out=out_ap[c], in_=NL[:])

    _strip_syncs(nc)
    return nc


def _strip_syncs(nc):
    """walrus on this stack supports a single sync-wait slot per instruction.
    Reduce each instruction's wait list via a vector-clock pass: walking the
    scheduled program order, every engine accumulates knowledge of semaphore
    values - from its own queue position, from waits it has already performed,
    and transitively from the producer's knowledge snapshot at the awaited
    update.  A wait already implied by that knowledge is dropped.  Kernel-tail
    drains keep only their DMA wait (the per-engine drain + EVSEM butterfly
    that follows enforces engine completion)."""
    import bass_rust

    eng_sem = {"EngineType.DVE": "DVE_", "EngineType.Pool": "Pool_",
               "EngineType.Activation": "Activation_", "EngineType.PE": "PE_",
               "EngineType.SP": "SP_"}
    know = {e: {} for e in eng_sem}          # engine -> {sem: value}
    sem_hist = {}                            # sem -> list of (cum_value, snapshot)
    sem_cum = {}                             # sem -> cumulative inc so far

    # Sems that are ever decremented (barrier gather sems) are not monotone;
    # leave their waits untouched and keep them out of the knowledge model.
    nonmono = set()
    for b in nc.m.functions[0].blocks:
        for inst in b.instructions:
            si = inst.sync_info
            if si is not None:
                for u in si.on_update:
                    if u.update_mode != "sem-inc":
                        nonmono.add(u.ant_name)

    def implied(k, sem, val):
        return k.get(sem, 0) >= val

    def learn(k, sem, val):
        if k.get(sem, 0) < val:
            k[sem] = val
        # transitively absorb the producer's snapshot at this update
        hist = sem_hist.get(sem)
        if hist:
            import bisect
            i = bisect.bisect_left([h[0] for h in hist], val)
            if i < len(hist):
                for s2, v2 in hist[i][1].items():
                    if k.get(s2, 0) < v2:
                        k[s2] = v2

    from concourse import mybir

    for b in nc.m.functions[0].blocks:
        new_instructions = []
        for inst in b.instructions:
            si = inst.sync_info
            eng = str(inst.engine)
            k = know.setdefault(eng, {})
            if si is not None:
                waits = list(si.on_wait)
                if type(inst).__name__ == "InstDrain" and len(waits) > 1:
                    dma = [w for w in waits if "DMA" in w.ant_name]
                    keep_w = dma[-1:] if dma else waits[:1]
                    for w in waits:
                        learn(k, w.ant_name, w.wait_value)
                else:
                    merged = {}
                    for w in waits:
                        if w.ant_name in nonmono:
                            merged[id(w)] = w
                        elif w.ant_name not in merged or \
                                merged[w.ant_name].wait_value < w.wait_value:
                            merged[w.ant_name] = w
                    keep_w = []
                    for w in merged.values():
                        if w.ant_name in nonmono:
                            keep_w.append(w)
                            continue
                        if not implied(k, w.ant_name, w.wait_value):
                            keep_w.append(w)
                        learn(k, w.ant_name, w.wait_value)
                    # walrus has one wait slot per instruction: hoist extra
                    # waits onto injected no-ops on the same engine
                    while len(keep_w) > 1:
                        w = keep_w.pop(0)
                        nop = mybir.InstNoOp(
                            name=f"{inst.name}_w{len(keep_w)}",
                            engine=inst.engine, ins=[], outs=[],
                            sync_info=bass_rust.SyncInfo(
                                on_wait=[w], on_update=[]))
                        new_instructions.append(nop)
                if len(keep_w) != len(waits):
                    inst.sync_info = bass_rust.SyncInfo(
                        on_wait=keep_w, on_update=list(si.on_update))
                    si = inst.sync_info
                for u in si.on_update:
                    if u.update_mode == "sem-inc" and u.ant_name not in nonmono:
                        name = u.ant_name
                        cum = sem_cum.get(name, 0) + u.update_value
                        sem_cum[name] = cum
                        # own-engine sems are implicitly ordered for later
                        # instructions on the same queue
                        pref = eng_sem.get(eng)
                        if pref and name.startswith(pref):
                            k[name] = max(k.get(name, 0), cum)
                        sem_hist.setdefault(name, []).append((cum, dict(k)))
            new_instructions.append(inst)
        if len(new_instructions) != len(b.instructions):
            b.instructions = new_instructions


def kernel(llr, max_iters):
    llr = np.ascontiguousarray(np.asarray(llr), dtype=np.float32)
    iters = int(np.asarray(max_iters))
    B = llr.shape[0]
    if iters <= 0:
        return llr.reshape(B, 1, 7).copy()

    from concourse.bass_utils import run_bass_kernel_spmd

    Bc = B // NCORES
    key = (Bc, iters)
    if key not in _CACHE:
        _CACHE[key] = _build(Bc, iters)
    nc = _CACHE[key]

    flat = llr.reshape(B, 7)
    in_maps = [{"llr": flat[i * Bc:(i + 1) * Bc]} for i in range(NCORES)]
    res = run_bass_kernel_spmd(nc, in_maps, core_ids=list(range(NCORES)))
    out = np.concatenate([np.asarray(r["out"]) for r in res.results], axis=0)
    return out.reshape(B, 1, 7)


# revision 23
# speedup vs baseline: 2.4121x; 1.1078x over previous
"""LDPC belief-propagation kernel for Trainium2 (8 NeuronCores, data-parallel).

Math (per batch row, H fixed [3,7], 12 edges):
  t_e   = tanh(m_e / 2)                       (signed!)
  u_e   = prod_{e' in check c, e' != e} t_e'  (signed leave-one-out product)
  c2v_e = 2 atanh(u_e) = ln(1+u) - ln(1-u)    (signed, sign handled for free)
  new_llr_v = llr_v + sum_{c contains v} c2v_{c,v}
  m'_e  = new_llr_v - c2v_e
Only Tanh/Ln tables; the sign pipeline of the classic phi/phi formulation
disappears because the tanh products carry signs natively.

Edge layout is role-major per 12-slot group: [s0 s1 s2 | a0 a1 a2 | b0 b1 b2 |
d0 d1 d2] where s_c is the check's degree-1-variable edge (v0,v1,v3 - their
messages never change), d_c is v6's edge in check c, and (a, b) =
((v2c0, v2c1, v4c2), (v4c0, v5c1, v5c2)).  This makes every structural op a
single strided instruction:
  Q[k]   = T[k] * T[k+6]         (k=0..5: pair products (s*b, a*d) per check)
  U[3..8]  = T[(9,10,11,0,1,2)] * Q[0..5]   (loo for roles a, b)
  U[9..11] = T[3..5] * Q[0..2]              (loo for role d)
  M'[deg2 six edges] = LB6 + CV[partner]    (partner = pair-swap view)
  M'[d]  = (c2v_d-sum + llr_6) - CV[d]      (v6 leave-one-out via total sum)
Batch is split into 2 chunks so ACT (Tanh/Ln/Ln), DVE (products, M-updates)
and Pool (CV subtract, v6 sums) pipeline across chunks.
"""

import numpy as np

_CACHE = {}

NCORES = 8
P = 128      # partitions
CHUNKS = 2   # batch sub-chunks per core (pipeline depth)

# guard so ln(1 -+ 0.99999988*u) stays finite (>= ~1.2e-7) even at u = -+1
LNSCALE = 0.99999988

# (start, half-period, partA->partB gap) ns hints for the list scheduler
_SCHED = (3800, 3700, 3400)


def _build(Bc, iters):
    import contextlib

    import concourse.bass as bass
    import concourse.tile as tile
    from concourse import mybir
    from concourse.alu_op_type import AluOpType as Op

    F = mybir.ActivationFunctionType
    W = Bc // P // CHUNKS  # free columns per partition per chunk
    f32 = mybir.dt.float32

    f16 = mybir.dt.float16
    nc = bass.Bass("TRN2", target_bir_lowering=False, debug=False,
                   num_devices=1)
    llr_d = nc.dram_tensor("llr", [Bc, 7], f32, kind="ExternalInput")
    out_d = nc.dram_tensor("out", [Bc, 7], f32, kind="ExternalOutput")

    def sub(t, off, dims):
        a = t[:] if callable(getattr(t, "__getitem__", None)) else t
        return bass.AP(tensor=a.tensor, offset=a.offset + off,
                       ap=[list(a.ap[0])] + [list(d) for d in dims])

    with tile.TileContext(nc) as tc:
        ctx = contextlib.ExitStack()
        with ctx:
            keep = ctx.enter_context(tc.tile_pool(name="keep", bufs=1))
            work = ctx.enter_context(tc.tile_pool(name="work", bufs=2))

            def K(name, k, dt=f32):
                return keep.tile([P, W * k], dt, tag=name, name=name)

            # per-chunk persistent state
            LLs = [K(f"LL{c}", 7) for c in range(CHUNKS)]    # llr, natural v order
            LBs = [K(f"LB{c}", 6, f16) for c in range(CHUNKS)]    # llr bcast for deg2 edges
            L6s = [K(f"L6{c}", 3, f16) for c in range(CHUNKS)]    # llr6 bcast for v6 edges
            Ts  = [K(f"T{c}", 12) for c in range(CHUNKS)]    # tanh(m/2) per edge
            Ms  = [K(f"M{c}", 9, f16) for c in range(CHUNKS)]     # dyn messages (T slots 3..11)
            NLs = [K(f"NL{c}", 7) for c in range(CHUNKS)]    # output llr

            act = nc.scalar.activation
            vec = nc.vector
            gps = nc.gpsimd

            llr_ap = llr_d.ap().rearrange("(c p w) v -> c p (w v)", c=CHUNKS, p=P)
            out_ap = out_d.ap().rearrange("(c p w) v -> c p (w v)", c=CHUNKS, p=P)

            for c in range(CHUNKS):
                eng = nc.sync if c == 0 else nc.gpsimd
                eng.dma_start(out=LLs[c][:], in_=llr_ap[c])

            def v7(t, off, *dims):
                return sub(t, off, [[7, W]] + [list(d) for d in dims])

            def v12(t, off, *dims):
                return sub(t, off, [[12, W]] + [list(d) for d in dims])

            def v9(t, off, *dims):
                return sub(t, off, [[9, W]] + [list(d) for d in dims])

            def v6(t, off, *dims):
                return sub(t, off, [[6, W]] + [list(d) for d in dims])

            def v3(t, off, *dims):
                return sub(t, off, [[3, W]] + [list(d) for d in dims])

            def v1(t, off, *dims):
                return sub(t, off, [[1, W]] + [list(d) for d in dims])

            state = [{} for _ in range(CHUNKS)]

            def partA(c, it):
                """tanh + products: T, Q, U."""
                LL, LB, T, M = LLs[c], LBs[c], Ts[c], Ms[c]
                last = (it == iters - 1)
                Q = work.tile([P, W * 6], f32, tag="Q", name="Q")
                U = work.tile([P, W * 12], f32, tag="U", name="U")
                state[c] = {"Q": Q, "U": U}

                if it == 0:
                    # t = tanh(llr/2) once, scatter to role-major edge slots
                    TL = work.tile([P, W * 7], f32, tag="TL", name="TL")
                    act(TL[:], LL[:], F.Tanh, scale=0.5)
                    vec.tensor_copy(v12(T, 0, [1, 2]), v7(TL, 0, [1, 2]))
                    vec.tensor_copy(v12(T, 2, [1, 1]), v7(TL, 3, [1, 1]))
                    vec.tensor_copy(v12(T, 3, [1, 2]), v7(TL, 2, [0, 2]))
                    vec.tensor_copy(v12(T, 5, [1, 2]), v7(TL, 4, [0, 2]))
                    vec.tensor_copy(v12(T, 7, [1, 2]), v7(TL, 5, [0, 2]))
                    vec.tensor_copy(v12(T, 9, [1, 3]), v7(TL, 6, [0, 3]))
                    # llr broadcast for the six deg-2 edges (a|b role order)
                    gps.tensor_copy(v6(LB, 0, [1, 2]), v7(LL, 2, [0, 2]))
                    gps.tensor_copy(v6(LB, 2, [1, 4]), v7(LL, 4, [1, 2], [0, 2]))
                    gps.tensor_copy(v3(L6s[c], 0, [1, 3]), v7(LL, 6, [0, 3]))
                else:
                    act(v12(T, 3, [1, 9]), M[:], F.Tanh, scale=0.5)

                # pair products and signed leave-one-out products
                vec.tensor_tensor(Q[:], v12(T, 0, [1, 6]),
                                  v12(T, 6, [1, 6]), Op.mult)
                vec.tensor_tensor(v12(U, 3, [1, 6]),
                                  v12(T, 9, [-9, 2], [1, 3]),
                                  v6(Q, 0, [1, 6]), Op.mult)
                gps.tensor_tensor(v12(U, 9, [1, 3]), v12(T, 3, [1, 3]),
                                  v6(Q, 0, [1, 3]), Op.mult)
                if last:
                    gps.tensor_tensor(v12(U, 0, [1, 3]), v12(T, 6, [1, 3]),
                                      v6(Q, 3, [1, 3]), Op.mult)

            def partB(c, it):
                """c2v + message/new-llr update."""
                LL, LB, M, NL = LLs[c], LBs[c], Ms[c], NLs[c]
                last = (it == iters - 1)
                U = state[c]["U"]
                LP = work.tile([P, W * 12], f16, tag="LP", name="LP")
                LM = work.tile([P, W * 12], f16, tag="LM", name="LM")
                CV = work.tile([P, W * 12], f16, tag="CV", name="CV")

                off, n = (0, 9) if last else (3, 6)
                # c2v = ln(1+u) - ln(1-u), guarded away from ln(0)
                act(v12(LP, off, [1, n + 3]), v12(U, off, [1, n + 3]), F.Ln,
                    bias=1.0, scale=LNSCALE)
                act(v12(LM, off, [1, n + 3]), v12(U, off, [1, n + 3]), F.Ln,
                    bias=1.0, scale=-LNSCALE)
                vec.tensor_tensor(v12(CV, off, [1, n + 3]),
                                  v12(LP, off, [1, n + 3]),
                                  v12(LM, off, [1, n + 3]), Op.subtract)

                # v6 leave-one-out sums of the d-role c2vs, depth 2:
                # X[0]=c10+c11  X[1]=c9+c11  X[2]=c9+c10
                X = work.tile([P, W * 3], f16, tag="X", name="X")
                vec.tensor_tensor(v3(X, 0, [1, 2]), v12(CV, 10, [-1, 2]),
                                  v12(CV, 11, [0, 2]), Op.add)
                gps.tensor_tensor(v3(X, 2, [1, 1]), v12(CV, 9, [1, 1]),
                                  v12(CV, 10, [1, 1]), Op.add)

                if not last:
                    # m' for the six deg-2 edges: llr + partner c2v
                    vec.tensor_tensor(v9(M, 0, [1, 6]), v6(LB, 0, [1, 6]),
                                      v12(CV, 4, [2, 3], [-1, 2]), Op.add)
                    # m' for v6 edges: llr6 + sum of the other two c2v_d
                    vec.tensor_tensor(v9(M, 6, [1, 3]), v3(L6s[c], 0, [1, 3]),
                                      v3(X, 0, [1, 3]), Op.add)
                else:
                    # new_llr in natural variable order
                    SP = work.tile([P, W * 3], f32, tag="SP", name="SP")
                    gps.tensor_tensor(v7(NL, 0, [1, 2]), v7(LL, 0, [1, 2]),
                                      v12(CV, 0, [1, 2]), Op.add)
                    gps.tensor_tensor(v7(NL, 3, [1, 1]), v7(LL, 3, [1, 1]),
                                      v12(CV, 2, [1, 1]), Op.add)
                    vec.tensor_tensor(SP[:], v12(CV, 3, [2, 3]),
                                      v12(CV, 4, [2, 3]), Op.add)
                    vec.tensor_tensor(v7(NL, 2, [1, 1]), v7(LL, 2, [1, 1]),
                                      v3(SP, 0, [1, 1]), Op.add)
                    vec.tensor_tensor(v7(NL, 4, [1, 2]), v7(LL, 4, [1, 2]),
                                      v3(SP, 1, [1, 2]), Op.add)
                    S1 = work.tile([P, W], f32, tag="S1", name="S1")
                    vec.tensor_tensor(S1[:], v3(X, 2, [1, 1]),
                                      v12(CV, 11, [1, 1]), Op.add)
                    vec.tensor_tensor(v7(NL, 6, [1, 1]), S1[:],
                                      v7(LL, 6, [1, 1]), Op.add)
                    half = [[7, W // 2]]
                    lo = bass.AP(tensor=NL[:].tensor, offset=NL[:].offset,
                                 ap=[list(NL[:].ap[0])] + half + [[1, 7]])
                    hi = bass.AP(tensor=NL[:].tensor,
                                 offset=NL[:].offset + (W // 2) * 7,
                                 ap=[list(NL[:].ap[0])] + half + [[1, 7]])
                    olo = out_ap[c][:, : (W // 2) * 7]
                    ohi = out_ap[c][:, (W // 2) * 7:]
                    (nc.sync if c == 0 else nc.gpsimd).dma_start(
                        out=olo, in_=lo)
                    (nc.gpsimd if c == 0 else nc.sync).dma_start(
                        out=ohi, in_=hi)

            # software-pipelined schedule: chunk 1 runs half an iteration
            # behind chunk 0 so each chunk's ACT phase (Tanh / Ln Ln) overlaps
            # the other chunk's vector phase (products / updates).  The
            # wait-until timestamps steer the Tile list scheduler into that
            # stagger; they are lower bounds only, data deps still rule.
            S0, HALF, GAP = _SCHED
            for it in range(iters):
                for c in range(CHUNKS):
                    tA = S0 + (2 * it + c) * HALF
                    with tc.tile_wait_until(tA / 1e6, enable=tA > 0):
                        partA(c, it)
                    with tc.tile_wait_until((tA + GAP) / 1e6):
                        partB(c, it)

    _strip_syncs(nc)
    return nc


def _strip_syncs(nc):
    """walrus on this stack supports a single sync-wait slot per instruction.
    Reduce each instruction's wait list via a vector-clock pass: walking the
    scheduled program order, every engine accumulates knowledge of semaphore
    values - from its own queue position, from waits it has already performed,
    and transitively from the producer's knowledge snapshot at the awaited
    update.  A wait already implied by that knowledge is dropped.  Kernel-tail
    drains keep only their DMA wait (the per-engine drain + EVSEM butterfly
    that follows enforces engine completion)."""
    import bass_rust

    eng_sem = {"EngineType.DVE": "DVE_", "EngineType.Pool": "Pool_",
               "EngineType.Activation": "Activation_", "EngineType.PE": "PE_",
               "EngineType.SP": "SP_"}
    know = {e: {} for e in eng_sem}          # engine -> {sem: value}
    sem_hist = {}                            # sem -> list of (cum_value, snapshot)
    sem_cum = {}                             # sem -> cumulative inc so far

    # Sems that are ever decremented (barrier gather sems) are not monotone;
    # leave their waits untouched and keep them out of the knowledge model.
    nonmono = set()
    for b in nc.m.functions[0].blocks:
        for inst in b.instructions:
            si = inst.sync_info
            if si is not None:
                for u in si.on_update:
                    if u.update_mode != "sem-inc":
                        nonmono.add(u.ant_name)

    def implied(k, sem, val):
        return k.get(sem, 0) >= val

    def learn(k, sem, val):
        if k.get(sem, 0) < val:
            k[sem] = val
        # transitively absorb the producer's snapshot at this update
        hist = sem_hist.get(sem)
        if hist:
            import bisect
            i = bisect.bisect_left([h[0] for h in hist], val)
            if i < len(hist):
                for s2, v2 in hist[i][1].items():
                    if k.get(s2, 0) < v2:
                        k[s2] = v2

    from concourse import mybir

    for b in nc.m.functions[0].blocks:
        new_instructions = []
        for inst in b.instructions:
            si = inst.sync_info
            eng = str(inst.engine)
            k = know.setdefault(eng, {})
            if si is not None:
                waits = list(si.on_wait)
                if type(inst).__name__ == "InstDrain" and len(waits) > 1:
                    dma = [w for w in waits if "DMA" in w.ant_name]
                    keep_w = dma[-1:] if dma else waits[:1]
                    for w in waits:
                        learn(k, w.ant_name, w.wait_value)
                else:
                    merged = {}
                    for w in waits:
                        if w.ant_name in nonmono:
                            merged[id(w)] = w
                        elif w.ant_name not in merged or \
                                merged[w.ant_name].wait_value < w.wait_value:
                            merged[w.ant_name] = w
                    keep_w = []
                    for w in merged.values():
                        if w.ant_name in nonmono:
                            keep_w.append(w)
                            continue
                        if not implied(k, w.ant_name, w.wait_value):
                            keep_w.append(w)
                        learn(k, w.ant_name, w.wait_value)
                    # walrus has one wait slot per instruction: hoist extra
                    # waits onto injected no-ops on the same engine
                    while len(keep_w) > 1:
                        w = keep_w.pop(0)
                        nop = mybir.InstNoOp(
                            name=f"{inst.name}_w{len(keep_w)}",
                            engine=inst.engine, ins=[], outs=[],
                            sync_info=bass_rust.SyncInfo(
                                on_wait=[w], on_update=[]))
                        new_instructions.append(nop)
                if len(keep_w) != len(waits):
                    inst.sync_info = bass_rust.SyncInfo(
                        on_wait=keep_w, on_update=list(si.on_update))
                    si = inst.sync_info
                for u in si.on_update:
                    if u.update_mode == "sem-inc" and u.ant_name not in nonmono:
                        name = u.ant_name
                        cum = sem_cum.get(name, 0) + u.update_value
                        sem_cum[name] = cum
                        # own-engine sems are implicitly ordered for later
                        # instructions on the same queue
                        pref = eng_sem.get(eng)
                        if pref and name.startswith(pref):
                            k[name] = max(k.get(name, 0), cum)
                        sem_hist.setdefault(name, []).append((cum, dict(k)))
            new_instructions.append(inst)
        if len(new_instructions) != len(b.instructions):
            b.instructions = new_instructions


def kernel(llr, max_iters):
    llr = np.ascontiguousarray(np.asarray(llr), dtype=np.float32)
    iters = int(np.asarray(max_iters))
    B = llr.shape[0]
    if iters <= 0:
        return llr.reshape(B, 1, 7).copy()

    from concourse.bass_utils import run_bass_kernel_spmd

    Bc = B // NCORES
    key = (Bc, iters)
    if key not in _CACHE:
        _CACHE[key] = _build(Bc, iters)
    nc = _CACHE[key]

    flat = llr.reshape(B, 7)
    in_maps = [{"llr": flat[i * Bc:(i + 1) * Bc]} for i in range(NCORES)]
    res = run_bass_kernel_spmd(nc, in_maps, core_ids=list(range(NCORES)))
    out = np.concatenate([np.asarray(r["out"]) for r in res.results], axis=0)
    return out.reshape(B, 1, 7)


# revision 29
# speedup vs baseline: 2.5868x; 1.0724x over previous
"""LDPC belief-propagation kernel for Trainium2 (8 NeuronCores, data-parallel).

Math (per batch row, H fixed [3,7], 12 edges):
  t_e   = tanh(m_e / 2)                       (signed!)
  u_e   = prod_{e' in check c, e' != e} t_e'  (signed leave-one-out product)
  c2v_e = 2 atanh(u_e) = ln(1+u) - ln(1-u)    (signed, sign handled for free)
  new_llr_v = llr_v + sum_{c contains v} c2v_{c,v}
  m'_e  = new_llr_v - c2v_e
Only Tanh/Ln tables; the sign pipeline of the classic phi/phi formulation
disappears because the tanh products carry signs natively.

Edge layout is role-major per 12-slot group: [s0 s1 s2 | a0 a1 a2 | b0 b1 b2 |
d0 d1 d2] where s_c is the check's degree-1-variable edge (v0,v1,v3 - their
messages never change), d_c is v6's edge in check c, and (a, b) =
((v2c0, v2c1, v4c2), (v4c0, v5c1, v5c2)).  This makes every structural op a
single strided instruction:
  Q[k]   = T[k] * T[k+6]         (k=0..5: pair products (s*b, a*d) per check)
  U[3..8]  = T[(9,10,11,0,1,2)] * Q[0..5]   (loo for roles a, b)
  U[9..11] = T[3..5] * Q[0..2]              (loo for role d)
  M'[deg2 six edges] = LB6 + CV[partner]    (partner = pair-swap view)
  M'[d]  = (c2v_d-sum + llr_6) - CV[d]      (v6 leave-one-out via total sum)
Batch is split into 2 chunks so ACT (Tanh/Ln/Ln), DVE (products, M-updates)
and Pool (CV subtract, v6 sums) pipeline across chunks.
"""

import numpy as np

_CACHE = {}

NCORES = 8
P = 128      # partitions
CHUNKS = 3   # batch sub-chunks per core (pipeline depth)

# guard so ln(1 -+ 0.99999988*u) stays finite (>= ~1.2e-7) even at u = -+1
LNSCALE = 0.99999988

# (start, half-period, partA->partB gap) ns hints for the list scheduler
_SCHED = (0, 0, 0)


def _build(Bc, iters):
    import contextlib

    import concourse.bass as bass
    import concourse.tile as tile
    from concourse import mybir
    from concourse.alu_op_type import AluOpType as Op

    F = mybir.ActivationFunctionType
    Wtot = Bc // P
    base, rem = divmod(Wtot, CHUNKS)
    Ws = [base + (1 if i < rem else 0) for i in range(CHUNKS)]
    f32 = mybir.dt.float32

    f16 = mybir.dt.float16
    nc = bass.Bass("TRN2", target_bir_lowering=False, debug=False,
                   num_devices=1)
    llr_d = nc.dram_tensor("llr", [Bc, 7], f32, kind="ExternalInput")
    out_d = nc.dram_tensor("out", [Bc, 7], f32, kind="ExternalOutput")

    def sub(t, off, dims):
        a = t[:] if callable(getattr(t, "__getitem__", None)) else t
        return bass.AP(tensor=a.tensor, offset=a.offset + off,
                       ap=[list(a.ap[0])] + [list(d) for d in dims])

    with tile.TileContext(nc) as tc:
        ctx = contextlib.ExitStack()
        with ctx:
            keep = ctx.enter_context(tc.tile_pool(name="keep", bufs=1))
            work = ctx.enter_context(tc.tile_pool(name="work", bufs=2))

            def K(name, c, k, dt=f32):
                return keep.tile([P, Ws[c] * k], dt, tag=name, name=name)

            # per-chunk persistent state
            LLs = [K(f"LL{c}", c, 7) for c in range(CHUNKS)]    # llr, natural v order
            LBs = [K(f"LB{c}", c, 6, f16) for c in range(CHUNKS)]   # llr bcast, deg2 edges
            L6s = [K(f"L6{c}", c, 3, f16) for c in range(CHUNKS)]   # llr6 bcast, v6 edges
            Ts  = [K(f"T{c}", c, 12) for c in range(CHUNKS)]    # tanh(m/2) per edge
            Ms  = [K(f"M{c}", c, 9, f16) for c in range(CHUNKS)]    # dyn messages
            NLs = [K(f"NL{c}", c, 7) for c in range(CHUNKS)]    # output llr

            act = nc.scalar.activation
            vec = nc.vector
            gps = nc.gpsimd

            def dram_view(t, c, w0, nw):
                # [P, nw*7] window of chunk c: rows base_c + p*Ws[c] + w
                a = t.ap()
                off = (P * sum(Ws[:c]) + w0) * 7
                return bass.AP(tensor=a.tensor, offset=a.offset + off,
                               ap=[[Ws[c] * 7, P], [1, nw * 7]])

            for c in range(CHUNKS):
                eng = nc.sync if c == 0 else nc.gpsimd
                eng.dma_start(out=LLs[c][:], in_=dram_view(llr_d, c, 0, Ws[c]))

            cur = {"W": Ws[0]}

            def v7(t, off, *dims):
                return sub(t, off, [[7, cur["W"]]] + [list(d) for d in dims])

            def v12(t, off, *dims):
                return sub(t, off, [[12, cur["W"]]] + [list(d) for d in dims])

            def v9(t, off, *dims):
                return sub(t, off, [[9, cur["W"]]] + [list(d) for d in dims])

            def v6(t, off, *dims):
                return sub(t, off, [[6, cur["W"]]] + [list(d) for d in dims])

            def v3(t, off, *dims):
                return sub(t, off, [[3, cur["W"]]] + [list(d) for d in dims])

            state = [{} for _ in range(CHUNKS)]

            def partA(c, it):
                """tanh + products: T, Q, U."""
                LL, LB, T, M = LLs[c], LBs[c], Ts[c], Ms[c]
                last = (it == iters - 1)
                W = Ws[c]
                cur["W"] = W
                Q = work.tile([P, W * 6], f32, tag=f"Q{c}", name="Q")
                U = work.tile([P, W * 12], f32, tag=f"U{c}", name="U")
                state[c] = {"Q": Q, "U": U}

                if it == 0:
                    # t = tanh(llr/2) once, scatter to role-major edge slots
                    TL = work.tile([P, W * 7], f32, tag=f"TL{c}", name="TL")
                    act(TL[:], LL[:], F.Tanh, scale=0.5)
                    vec.tensor_copy(v12(T, 0, [1, 2]), v7(TL, 0, [1, 2]))
                    vec.tensor_copy(v12(T, 2, [1, 1]), v7(TL, 3, [1, 1]))
                    vec.tensor_copy(v12(T, 3, [1, 2]), v7(TL, 2, [0, 2]))
                    vec.tensor_copy(v12(T, 5, [1, 2]), v7(TL, 4, [0, 2]))
                    vec.tensor_copy(v12(T, 7, [1, 2]), v7(TL, 5, [0, 2]))
                    vec.tensor_copy(v12(T, 9, [1, 3]), v7(TL, 6, [0, 3]))
                    # llr broadcast for the six deg-2 edges (a|b role order)
                    gps.tensor_copy(v6(LB, 0, [1, 2]), v7(LL, 2, [0, 2]))
                    gps.tensor_copy(v6(LB, 2, [1, 4]), v7(LL, 4, [1, 2], [0, 2]))
                    gps.tensor_copy(v3(L6s[c], 0, [1, 3]), v7(LL, 6, [0, 3]))
                else:
                    act(v12(T, 3, [1, 9]), M[:], F.Tanh, scale=0.5)

                # pair products and signed leave-one-out products
                vec.tensor_tensor(Q[:], v12(T, 0, [1, 6]),
                                  v12(T, 6, [1, 6]), Op.mult)
                vec.tensor_tensor(v12(U, 3, [1, 6]),
                                  v12(T, 9, [-9, 2], [1, 3]),
                                  v6(Q, 0, [1, 6]), Op.mult)
                gps.tensor_tensor(v12(U, 9, [1, 3]), v12(T, 3, [1, 3]),
                                  v6(Q, 0, [1, 3]), Op.mult)
                if last:
                    gps.tensor_tensor(v12(U, 0, [1, 3]), v12(T, 6, [1, 3]),
                                      v6(Q, 3, [1, 3]), Op.mult)

            def partB(c, it):
                """c2v + message/new-llr update."""
                LL, LB, M, NL = LLs[c], LBs[c], Ms[c], NLs[c]
                last = (it == iters - 1)
                W = Ws[c]
                cur["W"] = W
                U = state[c]["U"]
                LP = work.tile([P, W * 12], f16, tag=f"LP{c}", name="LP")
                LM = work.tile([P, W * 12], f16, tag=f"LM{c}", name="LM")
                CV = work.tile([P, W * 12], f16, tag=f"CV{c}", name="CV")

                off, n = (0, 9) if last else (3, 6)
                # c2v = ln(1+u) - ln(1-u), guarded away from ln(0)
                act(v12(LP, off, [1, n + 3]), v12(U, off, [1, n + 3]), F.Ln,
                    bias=1.0, scale=LNSCALE)
                act(v12(LM, off, [1, n + 3]), v12(U, off, [1, n + 3]), F.Ln,
                    bias=1.0, scale=-LNSCALE)
                vec.tensor_tensor(v12(CV, off, [1, n + 3]),
                                  v12(LP, off, [1, n + 3]),
                                  v12(LM, off, [1, n + 3]), Op.subtract)

                # v6 leave-one-out sums of the d-role c2vs, depth 2:
                # X[0]=c10+c11  X[1]=c9+c11  X[2]=c9+c10
                X = work.tile([P, W * 3], f16, tag=f"X{c}", name="X")
                vec.tensor_tensor(v3(X, 0, [1, 2]), v12(CV, 10, [-1, 2]),
                                  v12(CV, 11, [0, 2]), Op.add)
                gps.tensor_tensor(v3(X, 2, [1, 1]), v12(CV, 9, [1, 1]),
                                  v12(CV, 10, [1, 1]), Op.add)

                if not last:
                    # m' for the six deg-2 edges: llr + partner c2v
                    vec.tensor_tensor(v9(M, 0, [1, 6]), v6(LB, 0, [1, 6]),
                                      v12(CV, 4, [2, 3], [-1, 2]), Op.add)
                    # m' for v6 edges: llr6 + sum of the other two c2v_d
                    vec.tensor_tensor(v9(M, 6, [1, 3]), v3(L6s[c], 0, [1, 3]),
                                      v3(X, 0, [1, 3]), Op.add)
                else:
                    # new_llr in natural variable order
                    SP = work.tile([P, W * 3], f32, tag=f"SP{c}", name="SP")
                    gps.tensor_tensor(v7(NL, 0, [1, 2]), v7(LL, 0, [1, 2]),
                                      v12(CV, 0, [1, 2]), Op.add)
                    gps.tensor_tensor(v7(NL, 3, [1, 1]), v7(LL, 3, [1, 1]),
                                      v12(CV, 2, [1, 1]), Op.add)
                    vec.tensor_tensor(SP[:], v12(CV, 3, [2, 3]),
                                      v12(CV, 4, [2, 3]), Op.add)
                    vec.tensor_tensor(v7(NL, 2, [1, 1]), v7(LL, 2, [1, 1]),
                                      v3(SP, 0, [1, 1]), Op.add)
                    vec.tensor_tensor(v7(NL, 4, [1, 2]), v7(LL, 4, [1, 2]),
                                      v3(SP, 1, [1, 2]), Op.add)
                    S1 = work.tile([P, W], f32, tag=f"S1{c}", name="S1")
                    vec.tensor_tensor(S1[:], v3(X, 2, [1, 1]),
                                      v12(CV, 11, [1, 1]), Op.add)
                    vec.tensor_tensor(v7(NL, 6, [1, 1]), S1[:],
                                      v7(LL, 6, [1, 1]), Op.add)
                    wl = W // 2
                    wh = W - wl
                    lo = bass.AP(tensor=NL[:].tensor, offset=NL[:].offset,
                                 ap=[list(NL[:].ap[0])] + [[7, wl], [1, 7]])
                    hi = bass.AP(tensor=NL[:].tensor,
                                 offset=NL[:].offset + wl * 7,
                                 ap=[list(NL[:].ap[0])] + [[7, wh], [1, 7]])
                    (nc.sync if c % 2 == 0 else nc.gpsimd).dma_start(
                        out=dram_view(out_d, c, 0, wl), in_=lo)
                    (nc.gpsimd if c % 2 == 0 else nc.sync).dma_start(
                        out=dram_view(out_d, c, wl, wh), in_=hi)

            # software-pipelined schedule: chunk 1 runs half an iteration
            # behind chunk 0 so each chunk's ACT phase (Tanh / Ln Ln) overlaps
            # the other chunk's vector phase (products / updates).  The
            # wait-until timestamps steer the Tile list scheduler into that
            # stagger; they are lower bounds only, data deps still rule.
            S0, HALF, GAP = _SCHED
            for it in range(iters):
                for c in range(CHUNKS):
                    tA = S0 + (CHUNKS * it + c) * HALF
                    with tc.tile_wait_until(tA / 1e6, enable=tA > 0):
                        partA(c, it)
                    with tc.tile_wait_until((tA + GAP) / 1e6):
                        partB(c, it)

    _strip_syncs(nc)
    return nc


def _strip_syncs(nc):
    """walrus on this stack supports a single sync-wait slot per instruction.
    Reduce each instruction's wait list via a vector-clock pass: walking the
    scheduled program order, every engine accumulates knowledge of semaphore
    values - from its own queue position, from waits it has already performed,
    and transitively from the producer's knowledge snapshot at the awaited
    update.  A wait already implied by that knowledge is dropped.  Kernel-tail
    drains keep only their DMA wait (the per-engine drain + EVSEM butterfly
    that follows enforces engine completion)."""
    import bass_rust

    eng_sem = {"EngineType.DVE": "DVE_", "EngineType.Pool": "Pool_",
               "EngineType.Activation": "Activation_", "EngineType.PE": "PE_",
               "EngineType.SP": "SP_"}
    know = {e: {} for e in eng_sem}          # engine -> {sem: value}
    sem_hist = {}                            # sem -> list of (cum_value, snapshot)
    sem_cum = {}                             # sem -> cumulative inc so far

    # Sems that are ever decremented (barrier gather sems) are not monotone;
    # leave their waits untouched and keep them out of the knowledge model.
    nonmono = set()
    for b in nc.m.functions[0].blocks:
        for inst in b.instructions:
            si = inst.sync_info
            if si is not None:
                for u in si.on_update:
                    if u.update_mode != "sem-inc":
                        nonmono.add(u.ant_name)

    def implied(k, sem, val):
        return k.get(sem, 0) >= val

    def learn(k, sem, val):
        if k.get(sem, 0) < val:
            k[sem] = val
        # transitively absorb the producer's snapshot at this update
        hist = sem_hist.get(sem)
        if hist:
            import bisect
            i = bisect.bisect_left([h[0] for h in hist], val)
            if i < len(hist):
                for s2, v2 in hist[i][1].items():
                    if k.get(s2, 0) < v2:
                        k[s2] = v2

    from concourse import mybir

    for b in nc.m.functions[0].blocks:
        new_instructions = []
        for inst in b.instructions:
            si = inst.sync_info
            eng = str(inst.engine)
            k = know.setdefault(eng, {})
            if si is not None:
                waits = list(si.on_wait)
                if type(inst).__name__ == "InstDrain" and len(waits) > 1:
                    dma = [w for w in waits if "DMA" in w.ant_name]
                    keep_w = dma[-1:] if dma else waits[:1]
                    for w in waits:
                        learn(k, w.ant_name, w.wait_value)
                else:
                    merged = {}
                    for w in waits:
                        if w.ant_name in nonmono:
                            merged[id(w)] = w
                        elif w.ant_name not in merged or \
                                merged[w.ant_name].wait_value < w.wait_value:
                            merged[w.ant_name] = w
                    keep_w = []
                    for w in merged.values():
                        if w.ant_name in nonmono:
                            keep_w.append(w)
                            continue
                        if not implied(k, w.ant_name, w.wait_value):
                            keep_w.append(w)
                        learn(k, w.ant_name, w.wait_value)
                    # walrus has one wait slot per instruction: hoist extra
                    # waits onto injected no-ops on the same engine
                    while len(keep_w) > 1:
                        w = keep_w.pop(0)
                        nop = mybir.InstNoOp(
                            name=f"{inst.name}_w{len(keep_w)}",
                            engine=inst.engine, ins=[], outs=[],
                            sync_info=bass_rust.SyncInfo(
                                on_wait=[w], on_update=[]))
                        new_instructions.append(nop)
                if len(keep_w) != len(waits):
                    inst.sync_info = bass_rust.SyncInfo(
                        on_wait=keep_w, on_update=list(si.on_update))
                    si = inst.sync_info
                for u in si.on_update:
                    if u.update_mode == "sem-inc" and u.ant_name not in nonmono:
                        name = u.ant_name
                        cum = sem_cum.get(name, 0) + u.update_value
                        sem_cum[name] = cum
                        # own-engine sems are implicitly ordered for later
                        # instructions on the same queue
                        pref = eng_sem.get(eng)
                        if pref and name.startswith(pref):
                            k[name] = max(k.get(name, 0), cum)
                        sem_hist.setdefault(name, []).append((cum, dict(k)))
            new_instructions.append(inst)
        if len(new_instructions) != len(b.instructions):
            b.instructions = new_instructions


def kernel(llr, max_iters):
    llr = np.ascontiguousarray(np.asarray(llr), dtype=np.float32)
    iters = int(np.asarray(max_iters))
    B = llr.shape[0]
    if iters <= 0:
        return llr.reshape(B, 1, 7).copy()

    from concourse.bass_utils import run_bass_kernel_spmd

    Bc = B // NCORES
    key = (Bc, iters)
    if key not in _CACHE:
        _CACHE[key] = _build(Bc, iters)
    nc = _CACHE[key]

    flat = llr.reshape(B, 7)
    in_maps = [{"llr": flat[i * Bc:(i + 1) * Bc]} for i in range(NCORES)]
    res = run_bass_kernel_spmd(nc, in_maps, core_ids=list(range(NCORES)))
    out = np.concatenate([np.asarray(r["out"]) for r in res.results], axis=0)
    return out.reshape(B, 1, 7)


# revision 30
# speedup vs baseline: 2.5888x; 1.0008x over previous
"""LDPC belief-propagation kernel for Trainium2 (8 NeuronCores, data-parallel).

Math (per batch row, H fixed [3,7], 12 edges):
  t_e   = tanh(m_e / 2)                       (signed!)
  u_e   = prod_{e' in check c, e' != e} t_e'  (signed leave-one-out product)
  c2v_e = 2 atanh(u_e) = ln(1+u) - ln(1-u)    (signed, sign handled for free)
  new_llr_v = llr_v + sum_{c contains v} c2v_{c,v}
  m'_e  = new_llr_v - c2v_e
Only Tanh/Ln tables; the sign pipeline of the classic phi/phi formulation
disappears because the tanh products carry signs natively.

Edge layout is role-major per 12-slot group: [s0 s1 s2 | a0 a1 a2 | b0 b1 b2 |
d0 d1 d2] where s_c is the check's degree-1-variable edge (v0,v1,v3 - their
messages never change), d_c is v6's edge in check c, and (a, b) =
((v2c0, v2c1, v4c2), (v4c0, v5c1, v5c2)).  This makes every structural op a
single strided instruction:
  Q[k]   = T[k] * T[k+6]         (k=0..5: pair products (s*b, a*d) per check)
  U[3..8]  = T[(9,10,11,0,1,2)] * Q[0..5]   (loo for roles a, b)
  U[9..11] = T[3..5] * Q[0..2]              (loo for role d)
  M'[deg2 six edges] = LB6 + CV[partner]    (partner = pair-swap view)
  M'[d]  = (c2v_d-sum + llr_6) - CV[d]      (v6 leave-one-out via total sum)
Batch is split into 2 chunks so ACT (Tanh/Ln/Ln), DVE (products, M-updates)
and Pool (CV subtract, v6 sums) pipeline across chunks.
"""

import numpy as np

_CACHE = {}

NCORES = 8
P = 128      # partitions
CHUNKS = 3   # batch sub-chunks per core (pipeline depth)

# guard so ln(1 -+ 0.99999988*u) stays finite (>= ~1.2e-7) even at u = -+1
LNSCALE = 0.99999988

# (start, half-period, partA->partB gap) ns hints for the list scheduler
_SCHED = (0, 0, 0)


def _build(Bc, iters):
    import contextlib

    import concourse.bass as bass
    import concourse.tile as tile
    from concourse import mybir
    from concourse.alu_op_type import AluOpType as Op

    F = mybir.ActivationFunctionType
    Wtot = Bc // P
    base, rem = divmod(Wtot, CHUNKS)
    Ws = [base + (1 if i >= CHUNKS - rem else 0) for i in range(CHUNKS)]
    f32 = mybir.dt.float32

    f16 = mybir.dt.float16
    nc = bass.Bass("TRN2", target_bir_lowering=False, debug=False,
                   num_devices=1)
    llr_d = nc.dram_tensor("llr", [Bc, 7], f32, kind="ExternalInput")
    out_d = nc.dram_tensor("out", [Bc, 7], f32, kind="ExternalOutput")

    def sub(t, off, dims):
        a = t[:] if callable(getattr(t, "__getitem__", None)) else t
        return bass.AP(tensor=a.tensor, offset=a.offset + off,
                       ap=[list(a.ap[0])] + [list(d) for d in dims])

    with tile.TileContext(nc) as tc:
        ctx = contextlib.ExitStack()
        with ctx:
            keep = ctx.enter_context(tc.tile_pool(name="keep", bufs=1))
            work = ctx.enter_context(tc.tile_pool(name="work", bufs=2))

            def K(name, c, k, dt=f32):
                return keep.tile([P, Ws[c] * k], dt, tag=name, name=name)

            # per-chunk persistent state
            LLs = [K(f"LL{c}", c, 7) for c in range(CHUNKS)]    # llr, natural v order
            LBs = [K(f"LB{c}", c, 6, f16) for c in range(CHUNKS)]   # llr bcast, deg2 edges
            L6s = [K(f"L6{c}", c, 3, f16) for c in range(CHUNKS)]   # llr6 bcast, v6 edges
            Ts  = [K(f"T{c}", c, 12) for c in range(CHUNKS)]    # tanh(m/2) per edge
            Ms  = [K(f"M{c}", c, 9, f16) for c in range(CHUNKS)]    # dyn messages
            NLs = [K(f"NL{c}", c, 7) for c in range(CHUNKS)]    # output llr

            act = nc.scalar.activation
            vec = nc.vector
            gps = nc.gpsimd

            def dram_view(t, c, w0, nw):
                # [P, nw*7] window of chunk c: rows base_c + p*Ws[c] + w
                a = t.ap()
                off = (P * sum(Ws[:c]) + w0) * 7
                return bass.AP(tensor=a.tensor, offset=a.offset + off,
                               ap=[[Ws[c] * 7, P], [1, nw * 7]])

            for c in range(CHUNKS):
                eng = nc.sync if c == 0 else nc.gpsimd
                eng.dma_start(out=LLs[c][:], in_=dram_view(llr_d, c, 0, Ws[c]))

            cur = {"W": Ws[0]}

            def v7(t, off, *dims):
                return sub(t, off, [[7, cur["W"]]] + [list(d) for d in dims])

            def v12(t, off, *dims):
                return sub(t, off, [[12, cur["W"]]] + [list(d) for d in dims])

            def v9(t, off, *dims):
                return sub(t, off, [[9, cur["W"]]] + [list(d) for d in dims])

            def v6(t, off, *dims):
                return sub(t, off, [[6, cur["W"]]] + [list(d) for d in dims])

            def v3(t, off, *dims):
                return sub(t, off, [[3, cur["W"]]] + [list(d) for d in dims])

            state = [{} for _ in range(CHUNKS)]

            def partA(c, it):
                """tanh + products: T, Q, U."""
                LL, LB, T, M = LLs[c], LBs[c], Ts[c], Ms[c]
                last = (it == iters - 1)
                W = Ws[c]
                cur["W"] = W
                Q = work.tile([P, W * 6], f32, tag=f"Q{c}", name="Q")
                U = work.tile([P, W * 12], f32, tag=f"U{c}", name="U")
                state[c] = {"Q": Q, "U": U}

                if it == 0:
                    # t = tanh(llr/2) once, scatter to role-major edge slots
                    TL = work.tile([P, W * 7], f32, tag=f"TL{c}", name="TL")
                    act(TL[:], LL[:], F.Tanh, scale=0.5)
                    vec.tensor_copy(v12(T, 0, [1, 2]), v7(TL, 0, [1, 2]))
                    vec.tensor_copy(v12(T, 2, [1, 1]), v7(TL, 3, [1, 1]))
                    vec.tensor_copy(v12(T, 3, [1, 2]), v7(TL, 2, [0, 2]))
                    vec.tensor_copy(v12(T, 5, [1, 2]), v7(TL, 4, [0, 2]))
                    vec.tensor_copy(v12(T, 7, [1, 2]), v7(TL, 5, [0, 2]))
                    vec.tensor_copy(v12(T, 9, [1, 3]), v7(TL, 6, [0, 3]))
                    # llr broadcast for the six deg-2 edges (a|b role order)
                    gps.tensor_copy(v6(LB, 0, [1, 2]), v7(LL, 2, [0, 2]))
                    gps.tensor_copy(v6(LB, 2, [1, 4]), v7(LL, 4, [1, 2], [0, 2]))
                    gps.tensor_copy(v3(L6s[c], 0, [1, 3]), v7(LL, 6, [0, 3]))
                else:
                    act(v12(T, 3, [1, 9]), M[:], F.Tanh, scale=0.5)

                # pair products and signed leave-one-out products
                vec.tensor_tensor(Q[:], v12(T, 0, [1, 6]),
                                  v12(T, 6, [1, 6]), Op.mult)
                vec.tensor_tensor(v12(U, 3, [1, 6]),
                                  v12(T, 9, [-9, 2], [1, 3]),
                                  v6(Q, 0, [1, 6]), Op.mult)
                gps.tensor_tensor(v12(U, 9, [1, 3]), v12(T, 3, [1, 3]),
                                  v6(Q, 0, [1, 3]), Op.mult)
                if last:
                    gps.tensor_tensor(v12(U, 0, [1, 3]), v12(T, 6, [1, 3]),
                                      v6(Q, 3, [1, 3]), Op.mult)

            def partB(c, it):
                """c2v + message/new-llr update."""
                LL, LB, M, NL = LLs[c], LBs[c], Ms[c], NLs[c]
                last = (it == iters - 1)
                W = Ws[c]
                cur["W"] = W
                U = state[c]["U"]
                LP = work.tile([P, W * 12], f16, tag=f"LP{c}", name="LP")
                LM = work.tile([P, W * 12], f16, tag=f"LM{c}", name="LM")
                CV = work.tile([P, W * 12], f16, tag=f"CV{c}", name="CV")

                off, n = (0, 9) if last else (3, 6)
                # c2v = ln(1+u) - ln(1-u), guarded away from ln(0)
                act(v12(LP, off, [1, n + 3]), v12(U, off, [1, n + 3]), F.Ln,
                    bias=1.0, scale=LNSCALE)
                act(v12(LM, off, [1, n + 3]), v12(U, off, [1, n + 3]), F.Ln,
                    bias=1.0, scale=-LNSCALE)
                vec.tensor_tensor(v12(CV, off, [1, n + 3]),
                                  v12(LP, off, [1, n + 3]),
                                  v12(LM, off, [1, n + 3]), Op.subtract)

                # v6 leave-one-out sums of the d-role c2vs, depth 2:
                # X[0]=c10+c11  X[1]=c9+c11  X[2]=c9+c10
                X = work.tile([P, W * 3], f16, tag=f"X{c}", name="X")
                vec.tensor_tensor(v3(X, 0, [1, 2]), v12(CV, 10, [-1, 2]),
                                  v12(CV, 11, [0, 2]), Op.add)
                gps.tensor_tensor(v3(X, 2, [1, 1]), v12(CV, 9, [1, 1]),
                                  v12(CV, 10, [1, 1]), Op.add)

                if not last:
                    # m' for the six deg-2 edges: llr + partner c2v
                    vec.tensor_tensor(v9(M, 0, [1, 6]), v6(LB, 0, [1, 6]),
                                      v12(CV, 4, [2, 3], [-1, 2]), Op.add)
                    # m' for v6 edges: llr6 + sum of the other two c2v_d
                    vec.tensor_tensor(v9(M, 6, [1, 3]), v3(L6s[c], 0, [1, 3]),
                                      v3(X, 0, [1, 3]), Op.add)
                else:
                    # new_llr in natural variable order
                    SP = work.tile([P, W * 3], f32, tag=f"SP{c}", name="SP")
                    gps.tensor_tensor(v7(NL, 0, [1, 2]), v7(LL, 0, [1, 2]),
                                      v12(CV, 0, [1, 2]), Op.add)
                    gps.tensor_tensor(v7(NL, 3, [1, 1]), v7(LL, 3, [1, 1]),
                                      v12(CV, 2, [1, 1]), Op.add)
                    vec.tensor_tensor(SP[:], v12(CV, 3, [2, 3]),
                                      v12(CV, 4, [2, 3]), Op.add)
                    vec.tensor_tensor(v7(NL, 2, [1, 1]), v7(LL, 2, [1, 1]),
                                      v3(SP, 0, [1, 1]), Op.add)
                    vec.tensor_tensor(v7(NL, 4, [1, 2]), v7(LL, 4, [1, 2]),
                                      v3(SP, 1, [1, 2]), Op.add)
                    S1 = work.tile([P, W], f32, tag=f"S1{c}", name="S1")
                    vec.tensor_tensor(S1[:], v3(X, 2, [1, 1]),
                                      v12(CV, 11, [1, 1]), Op.add)
                    vec.tensor_tensor(v7(NL, 6, [1, 1]), S1[:],
                                      v7(LL, 6, [1, 1]), Op.add)
                    wl = W // 2
                    wh = W - wl
                    lo = bass.AP(tensor=NL[:].tensor, offset=NL[:].offset,
                                 ap=[list(NL[:].ap[0])] + [[7, wl], [1, 7]])
                    hi = bass.AP(tensor=NL[:].tensor,
                                 offset=NL[:].offset + wl * 7,
                                 ap=[list(NL[:].ap[0])] + [[7, wh], [1, 7]])
                    (nc.sync if c % 2 == 0 else nc.gpsimd).dma_start(
                        out=dram_view(out_d, c, 0, wl), in_=lo)
                    (nc.gpsimd if c % 2 == 0 else nc.sync).dma_start(
                        out=dram_view(out_d, c, wl, wh), in_=hi)

            # software-pipelined schedule: chunk 1 runs half an iteration
            # behind chunk 0 so each chunk's ACT phase (Tanh / Ln Ln) overlaps
            # the other chunk's vector phase (products / updates).  The
            # wait-until timestamps steer the Tile list scheduler into that
            # stagger; they are lower bounds only, data deps still rule.
            S0, HALF, GAP = _SCHED
            for it in range(iters):
                for c in range(CHUNKS):
                    tA = S0 + (CHUNKS * it + c) * HALF
                    with tc.tile_wait_until(tA / 1e6, enable=tA > 0):
                        partA(c, it)
                    with tc.tile_wait_until((tA + GAP) / 1e6):
                        partB(c, it)

    _strip_syncs(nc)
    return nc


def _strip_syncs(nc):
    """walrus on this stack supports a single sync-wait slot per instruction.
    Reduce each instruction's wait list via a vector-clock pass: walking the
    scheduled program order, every engine accumulates knowledge of semaphore
    values - from its own queue position, from waits it has already performed,
    and transitively from the producer's knowledge snapshot at the awaited
    update.  A wait already implied by that knowledge is dropped.  Kernel-tail
    drains keep only their DMA wait (the per-engine drain + EVSEM butterfly
    that follows enforces engine completion)."""
    import bass_rust

    eng_sem = {"EngineType.DVE": "DVE_", "EngineType.Pool": "Pool_",
               "EngineType.Activation": "Activation_", "EngineType.PE": "PE_",
               "EngineType.SP": "SP_"}
    know = {e: {} for e in eng_sem}          # engine -> {sem: value}
    sem_hist = {}                            # sem -> list of (cum_value, snapshot)
    sem_cum = {}                             # sem -> cumulative inc so far

    # Sems that are ever decremented (barrier gather sems) are not monotone;
    # leave their waits untouched and keep them out of the knowledge model.
    nonmono = set()
    for b in nc.m.functions[0].blocks:
        for inst in b.instructions:
            si = inst.sync_info
            if si is not None:
                for u in si.on_update:
                    if u.update_mode != "sem-inc":
                        nonmono.add(u.ant_name)

    def implied(k, sem, val):
        return k.get(sem, 0) >= val

    def learn(k, sem, val):
        if k.get(sem, 0) < val:
            k[sem] = val
        # transitively absorb the producer's snapshot at this update
        hist = sem_hist.get(sem)
        if hist:
            import bisect
            i = bisect.bisect_left([h[0] for h in hist], val)
            if i < len(hist):
                for s2, v2 in hist[i][1].items():
                    if k.get(s2, 0) < v2:
                        k[s2] = v2

    from concourse import mybir

    for b in nc.m.functions[0].blocks:
        new_instructions = []
        for inst in b.instructions:
            si = inst.sync_info
            eng = str(inst.engine)
            k = know.setdefault(eng, {})
            if si is not None:
                waits = list(si.on_wait)
                if type(inst).__name__ == "InstDrain" and len(waits) > 1:
                    dma = [w for w in waits if "DMA" in w.ant_name]
                    keep_w = dma[-1:] if dma else waits[:1]
                    for w in waits:
                        learn(k, w.ant_name, w.wait_value)
                else:
                    merged = {}
                    for w in waits:
                        if w.ant_name in nonmono:
                            merged[id(w)] = w
                        elif w.ant_name not in merged or \
                                merged[w.ant_name].wait_value < w.wait_value:
                            merged[w.ant_name] = w
                    keep_w = []
                    for w in merged.values():
                        if w.ant_name in nonmono:
                            keep_w.append(w)
                            continue
                        if not implied(k, w.ant_name, w.wait_value):
                            keep_w.append(w)
                        learn(k, w.ant_name, w.wait_value)
                    # walrus has one wait slot per instruction: hoist extra
                    # waits onto injected no-ops on the same engine
                    while len(keep_w) > 1:
                        w = keep_w.pop(0)
                        nop = mybir.InstNoOp(
                            name=f"{inst.name}_w{len(keep_w)}",
                            engine=inst.engine, ins=[], outs=[],
                            sync_info=bass_rust.SyncInfo(
                                on_wait=[w], on_update=[]))
                        new_instructions.append(nop)
                if len(keep_w) != len(waits):
                    inst.sync_info = bass_rust.SyncInfo(
                        on_wait=keep_w, on_update=list(si.on_update))
                    si = inst.sync_info
                for u in si.on_update:
                    if u.update_mode == "sem-inc" and u.ant_name not in nonmono:
                        name = u.ant_name
                        cum = sem_cum.get(name, 0) + u.update_value
                        sem_cum[name] = cum
                        # own-engine sems are implicitly ordered for later
                        # instructions on the same queue
                        pref = eng_sem.get(eng)
                        if pref and name.startswith(pref):
                            k[name] = max(k.get(name, 0), cum)
                        sem_hist.setdefault(name, []).append((cum, dict(k)))
            new_instructions.append(inst)
        if len(new_instructions) != len(b.instructions):
            b.instructions = new_instructions


def kernel(llr, max_iters):
    llr = np.ascontiguousarray(np.asarray(llr), dtype=np.float32)
    iters = int(np.asarray(max_iters))
    B = llr.shape[0]
    if iters <= 0:
        return llr.reshape(B, 1, 7).copy()

    from concourse.bass_utils import run_bass_kernel_spmd

    Bc = B // NCORES
    key = (Bc, iters)
    if key not in _CACHE:
        _CACHE[key] = _build(Bc, iters)
    nc = _CACHE[key]

    flat = llr.reshape(B, 7)
    in_maps = [{"llr": flat[i * Bc:(i + 1) * Bc]} for i in range(NCORES)]
    res = run_bass_kernel_spmd(nc, in_maps, core_ids=list(range(NCORES)))
    out = np.concatenate([np.asarray(r["out"]) for r in res.results], axis=0)
    return out.reshape(B, 1, 7)


# revision 36
# speedup vs baseline: 2.6078x; 1.0073x over previous
"""LDPC belief-propagation kernel for Trainium2 (8 NeuronCores, data-parallel).

Math (per batch row, H fixed [3,7], 12 edges):
  t_e   = tanh(m_e / 2)                       (signed!)
  u_e   = prod_{e' in check c, e' != e} t_e'  (signed leave-one-out product)
  c2v_e = 2 atanh(u_e) = ln(1+u) - ln(1-u)    (signed, sign handled for free)
  new_llr_v = llr_v + sum_{c contains v} c2v_{c,v}
  m'_e  = new_llr_v - c2v_e
Only Tanh/Ln tables; the sign pipeline of the classic phi/phi formulation
disappears because the tanh products carry signs natively.

Edge layout is role-major per 12-slot group: [s0 s1 s2 | a0 a1 a2 | b0 b1 b2 |
d0 d1 d2] where s_c is the check's degree-1-variable edge (v0,v1,v3 - their
messages never change), d_c is v6's edge in check c, and (a, b) =
((v2c0, v2c1, v4c2), (v4c0, v5c1, v5c2)).  This makes every structural op a
single strided instruction:
  Q[k]   = T[k] * T[k+6]         (k=0..5: pair products (s*b, a*d) per check)
  U[3..8]  = T[(9,10,11,0,1,2)] * Q[0..5]   (loo for roles a, b)
  U[9..11] = T[3..5] * Q[0..2]              (loo for role d)
  M'[deg2 six edges] = LB6 + CV[partner]    (partner = pair-swap view)
  M'[d]  = (c2v_d-sum + llr_6) - CV[d]      (v6 leave-one-out via total sum)
Batch is split into 2 chunks so ACT (Tanh/Ln/Ln), DVE (products, M-updates)
and Pool (CV subtract, v6 sums) pipeline across chunks.
"""

import numpy as np

_CACHE = {}

NCORES = 8
P = 128      # partitions
CHUNKS = 3   # batch sub-chunks per core (pipeline depth)

# guard so ln(1 -+ 0.99999988*u) stays finite (>= ~1.2e-7) even at u = -+1
LNSCALE = 0.99999988

# (start, half-period, partA->partB gap) ns hints for the list scheduler
_SCHED = (0, 0, 0)

# schedule hints deferring chunk setup copies (ns)
_CPY = (0, 9000, 11500)


def _build(Bc, iters):
    import contextlib

    import concourse.bass as bass
    import concourse.tile as tile
    from concourse import mybir
    from concourse.alu_op_type import AluOpType as Op

    F = mybir.ActivationFunctionType
    Wtot = Bc // P
    base, rem = divmod(Wtot, CHUNKS)
    Ws = [base + (1 if i >= CHUNKS - rem else 0) for i in range(CHUNKS)]
    f32 = mybir.dt.float32

    f16 = mybir.dt.float16
    nc = bass.Bass("TRN2", target_bir_lowering=False, debug=False,
                   num_devices=1)
    llr_d = nc.dram_tensor("llr", [Bc, 7], f32, kind="ExternalInput")
    out_d = nc.dram_tensor("out", [Bc, 7], f32, kind="ExternalOutput")

    def sub(t, off, dims):
        a = t[:] if callable(getattr(t, "__getitem__", None)) else t
        return bass.AP(tensor=a.tensor, offset=a.offset + off,
                       ap=[list(a.ap[0])] + [list(d) for d in dims])

    with tile.TileContext(nc) as tc:
        ctx = contextlib.ExitStack()
        with ctx:
            keep = ctx.enter_context(tc.tile_pool(name="keep", bufs=1))
            work = ctx.enter_context(tc.tile_pool(name="work", bufs=2))

            def K(name, c, k, dt=f32):
                return keep.tile([P, Ws[c] * k], dt, tag=name, name=name)

            # per-chunk persistent state
            LLs = [K(f"LL{c}", c, 7) for c in range(CHUNKS)]    # llr, natural v order
            LBs = [K(f"LB{c}", c, 6, f16) for c in range(CHUNKS)]   # llr bcast, deg2 edges
            L6s = [K(f"L6{c}", c, 3, f16) for c in range(CHUNKS)]   # llr6 bcast, v6 edges
            Ts  = [K(f"T{c}", c, 12) for c in range(CHUNKS)]    # tanh(m/2) per edge
            Ms  = [K(f"M{c}", c, 9, f16) for c in range(CHUNKS)]    # dyn messages
            NLs = [K(f"NL{c}", c, 7) for c in range(CHUNKS)]    # output llr

            act = nc.scalar.activation
            vec = nc.vector
            gps = nc.gpsimd

            def dram_view(t, c, w0, nw):
                # [P, nw*7] window of chunk c: rows base_c + p*Ws[c] + w
                a = t.ap()
                off = (P * sum(Ws[:c]) + w0) * 7
                return bass.AP(tensor=a.tensor, offset=a.offset + off,
                               ap=[[Ws[c] * 7, P], [1, nw * 7]])

            for c in range(CHUNKS):
                eng = nc.sync if c == 0 else nc.gpsimd
                eng.dma_start(out=LLs[c][:], in_=dram_view(llr_d, c, 0, Ws[c]))

            cur = {"W": Ws[0]}

            def v7(t, off, *dims):
                return sub(t, off, [[7, cur["W"]]] + [list(d) for d in dims])

            def v12(t, off, *dims):
                return sub(t, off, [[12, cur["W"]]] + [list(d) for d in dims])

            def v9(t, off, *dims):
                return sub(t, off, [[9, cur["W"]]] + [list(d) for d in dims])

            def v6(t, off, *dims):
                return sub(t, off, [[6, cur["W"]]] + [list(d) for d in dims])

            def v3(t, off, *dims):
                return sub(t, off, [[3, cur["W"]]] + [list(d) for d in dims])

            state = [{} for _ in range(CHUNKS)]

            def partA(c, it):
                """tanh + products: T, Q, U."""
                LL, LB, T, M = LLs[c], LBs[c], Ts[c], Ms[c]
                last = (it == iters - 1)
                W = Ws[c]
                cur["W"] = W
                Q = work.tile([P, W * 6], f32, tag=f"Q{c}", name="Q")
                U = work.tile([P, W * 12], f32, tag=f"U{c}", name="U")
                state[c] = {"Q": Q, "U": U}

                if it == 0:
                    # t = tanh(llr/2) once, scatter to role-major edge slots
                    TL = work.tile([P, W * 7], f32, tag=f"TL{c}", name="TL")
                    act(TL[:], LL[:], F.Tanh, scale=0.5)
                    vec.tensor_copy(v12(T, 0, [1, 2]), v7(TL, 0, [1, 2]))
                    vec.tensor_copy(v12(T, 2, [1, 1]), v7(TL, 3, [1, 1]))
                    vec.tensor_copy(v12(T, 3, [1, 2]), v7(TL, 2, [0, 2]))
                    vec.tensor_copy(v12(T, 5, [1, 2]), v7(TL, 4, [0, 2]))
                    vec.tensor_copy(v12(T, 7, [1, 2]), v7(TL, 5, [0, 2]))
                    vec.tensor_copy(v12(T, 9, [1, 3]), v7(TL, 6, [0, 3]))
                    # llr broadcast for the six deg-2 edges (a|b role order);
                    # deferred so they don't block the first chunks' U3 on Pool
                    with tc.tile_wait_until(_CPY[c] / 1e6, enable=_CPY[c] > 0):
                        gps.tensor_copy(v6(LB, 0, [1, 2]), v7(LL, 2, [0, 2]))
                        gps.tensor_copy(v6(LB, 2, [1, 4]),
                                        v7(LL, 4, [1, 2], [0, 2]))
                        gps.tensor_copy(v3(L6s[c], 0, [1, 3]),
                                        v7(LL, 6, [0, 3]))
                else:
                    act(v12(T, 3, [1, 9]), M[:], F.Tanh, scale=0.5)

                # pair products and signed leave-one-out products
                vec.tensor_tensor(Q[:], v12(T, 0, [1, 6]),
                                  v12(T, 6, [1, 6]), Op.mult)
                vec.tensor_tensor(v12(U, 3, [1, 6]),
                                  v12(T, 9, [-9, 2], [1, 3]),
                                  v6(Q, 0, [1, 6]), Op.mult)
                gps.tensor_tensor(v12(U, 9, [1, 3]), v12(T, 3, [1, 3]),
                                  v6(Q, 0, [1, 3]), Op.mult)
                if last:
                    gps.tensor_tensor(v12(U, 0, [1, 3]), v12(T, 6, [1, 3]),
                                      v6(Q, 3, [1, 3]), Op.mult)

            def partB(c, it):
                """c2v + message/new-llr update."""
                LL, LB, M, NL = LLs[c], LBs[c], Ms[c], NLs[c]
                last = (it == iters - 1)
                W = Ws[c]
                cur["W"] = W
                U = state[c]["U"]
                LP = work.tile([P, W * 12], f16, tag=f"LP{c}", name="LP")
                LM = work.tile([P, W * 12], f16, tag=f"LM{c}", name="LM")
                CV = work.tile([P, W * 12], f16, tag=f"CV{c}", name="CV")

                off, n = (0, 9) if last else (3, 6)
                # c2v = ln(1+u) - ln(1-u), guarded away from ln(0)
                act(v12(LP, off, [1, n + 3]), v12(U, off, [1, n + 3]), F.Ln,
                    bias=1.0, scale=LNSCALE)
                act(v12(LM, off, [1, n + 3]), v12(U, off, [1, n + 3]), F.Ln,
                    bias=1.0, scale=-LNSCALE)
                vec.tensor_tensor(v12(CV, off, [1, n + 3]),
                                  v12(LP, off, [1, n + 3]),
                                  v12(LM, off, [1, n + 3]), Op.subtract)

                # v6 leave-one-out sums of the d-role c2vs, depth 2:
                # X[0]=c10+c11  X[1]=c9+c11  X[2]=c9+c10
                X = work.tile([P, W * 3], f16, tag=f"X{c}", name="X")
                vec.tensor_tensor(v3(X, 0, [1, 2]), v12(CV, 10, [-1, 2]),
                                  v12(CV, 11, [0, 2]), Op.add)
                gps.tensor_tensor(v3(X, 2, [1, 1]), v12(CV, 9, [1, 1]),
                                  v12(CV, 10, [1, 1]), Op.add)

                if not last:
                    # m' for the six deg-2 edges: llr + partner c2v
                    vec.tensor_tensor(v9(M, 0, [1, 6]), v6(LB, 0, [1, 6]),
                                      v12(CV, 4, [2, 3], [-1, 2]), Op.add)
                    # m' for v6 edges: llr6 + sum of the other two c2v_d
                    vec.tensor_tensor(v9(M, 6, [1, 3]), v3(L6s[c], 0, [1, 3]),
                                      v3(X, 0, [1, 3]), Op.add)
                else:
                    # new_llr in natural variable order
                    SP = work.tile([P, W * 3], f32, tag=f"SP{c}", name="SP")
                    gps.tensor_tensor(v7(NL, 0, [1, 2]), v7(LL, 0, [1, 2]),
                                      v12(CV, 0, [1, 2]), Op.add)
                    gps.tensor_tensor(v7(NL, 3, [1, 1]), v7(LL, 3, [1, 1]),
                                      v12(CV, 2, [1, 1]), Op.add)
                    vec.tensor_tensor(SP[:], v12(CV, 3, [2, 3]),
                                      v12(CV, 4, [2, 3]), Op.add)
                    vec.tensor_tensor(v7(NL, 2, [1, 1]), v7(LL, 2, [1, 1]),
                                      v3(SP, 0, [1, 1]), Op.add)
                    vec.tensor_tensor(v7(NL, 4, [1, 2]), v7(LL, 4, [1, 2]),
                                      v3(SP, 1, [1, 2]), Op.add)
                    S1 = work.tile([P, W], f32, tag=f"S1{c}", name="S1")
                    vec.tensor_tensor(S1[:], v3(X, 2, [1, 1]),
                                      v12(CV, 11, [1, 1]), Op.add)
                    vec.tensor_tensor(v7(NL, 6, [1, 1]), S1[:],
                                      v7(LL, 6, [1, 1]), Op.add)
                    wl = W // 2
                    wh = W - wl
                    lo = bass.AP(tensor=NL[:].tensor, offset=NL[:].offset,
                                 ap=[list(NL[:].ap[0])] + [[7, wl], [1, 7]])
                    hi = bass.AP(tensor=NL[:].tensor,
                                 offset=NL[:].offset + wl * 7,
                                 ap=[list(NL[:].ap[0])] + [[7, wh], [1, 7]])
                    (nc.sync if c % 2 == 0 else nc.gpsimd).dma_start(
                        out=dram_view(out_d, c, 0, wl), in_=lo)
                    (nc.gpsimd if c % 2 == 0 else nc.sync).dma_start(
                        out=dram_view(out_d, c, wl, wh), in_=hi)

            # software-pipelined schedule: chunk 1 runs half an iteration
            # behind chunk 0 so each chunk's ACT phase (Tanh / Ln Ln) overlaps
            # the other chunk's vector phase (products / updates).  The
            # wait-until timestamps steer the Tile list scheduler into that
            # stagger; they are lower bounds only, data deps still rule.
            S0, HALF, GAP = _SCHED
            for it in range(iters):
                for c in range(CHUNKS):
                    tA = S0 + (CHUNKS * it + c) * HALF
                    with tc.tile_wait_until(tA / 1e6, enable=tA > 0):
                        partA(c, it)
                    with tc.tile_wait_until((tA + GAP) / 1e6):
                        partB(c, it)

    _strip_syncs(nc)
    return nc


def _strip_syncs(nc):
    """walrus on this stack supports a single sync-wait slot per instruction.
    Reduce each instruction's wait list via a vector-clock pass: walking the
    scheduled program order, every engine accumulates knowledge of semaphore
    values - from its own queue position, from waits it has already performed,
    and transitively from the producer's knowledge snapshot at the awaited
    update.  A wait already implied by that knowledge is dropped.  Kernel-tail
    drains keep only their DMA wait (the per-engine drain + EVSEM butterfly
    that follows enforces engine completion)."""
    import bass_rust

    eng_sem = {"EngineType.DVE": "DVE_", "EngineType.Pool": "Pool_",
               "EngineType.Activation": "Activation_", "EngineType.PE": "PE_",
               "EngineType.SP": "SP_"}
    know = {e: {} for e in eng_sem}          # engine -> {sem: value}
    sem_hist = {}                            # sem -> list of (cum_value, snapshot)
    sem_cum = {}                             # sem -> cumulative inc so far

    # Sems that are ever decremented (barrier gather sems) are not monotone;
    # leave their waits untouched and keep them out of the knowledge model.
    nonmono = set()
    for b in nc.m.functions[0].blocks:
        for inst in b.instructions:
            si = inst.sync_info
            if si is not None:
                for u in si.on_update:
                    if u.update_mode != "sem-inc":
                        nonmono.add(u.ant_name)

    def implied(k, sem, val):
        return k.get(sem, 0) >= val

    def learn(k, sem, val):
        if k.get(sem, 0) < val:
            k[sem] = val
        # transitively absorb the producer's snapshot at this update
        hist = sem_hist.get(sem)
        if hist:
            import bisect
            i = bisect.bisect_left([h[0] for h in hist], val)
            if i < len(hist):
                for s2, v2 in hist[i][1].items():
                    if k.get(s2, 0) < v2:
                        k[s2] = v2

    from concourse import mybir

    for b in nc.m.functions[0].blocks:
        new_instructions = []
        for inst in b.instructions:
            si = inst.sync_info
            eng = str(inst.engine)
            k = know.setdefault(eng, {})
            if si is not None:
                waits = list(si.on_wait)
                if type(inst).__name__ == "InstDrain" and len(waits) > 1:
                    dma = [w for w in waits if "DMA" in w.ant_name]
                    keep_w = dma[-1:] if dma else waits[:1]
                    for w in waits:
                        learn(k, w.ant_name, w.wait_value)
                else:
                    merged = {}
                    for w in waits:
                        if w.ant_name in nonmono:
                            merged[id(w)] = w
                        elif w.ant_name not in merged or \
                                merged[w.ant_name].wait_value < w.wait_value:
                            merged[w.ant_name] = w
                    keep_w = []
                    for w in merged.values():
                        if w.ant_name in nonmono:
                            keep_w.append(w)
                            continue
                        if not implied(k, w.ant_name, w.wait_value):
                            keep_w.append(w)
                        learn(k, w.ant_name, w.wait_value)
                    # walrus has one wait slot per instruction: hoist extra
                    # waits onto injected no-ops on the same engine
                    while len(keep_w) > 1:
                        w = keep_w.pop(0)
                        nop = mybir.InstNoOp(
                            name=f"{inst.name}_w{len(keep_w)}",
                            engine=inst.engine, ins=[], outs=[],
                            sync_info=bass_rust.SyncInfo(
                                on_wait=[w], on_update=[]))
                        new_instructions.append(nop)
                if len(keep_w) != len(waits):
                    inst.sync_info = bass_rust.SyncInfo(
                        on_wait=keep_w, on_update=list(si.on_update))
                    si = inst.sync_info
                for u in si.on_update:
                    if u.update_mode == "sem-inc" and u.ant_name not in nonmono:
                        name = u.ant_name
                        cum = sem_cum.get(name, 0) + u.update_value
                        sem_cum[name] = cum
                        # own-engine sems are implicitly ordered for later
                        # instructions on the same queue
                        pref = eng_sem.get(eng)
                        if pref and name.startswith(pref):
                            k[name] = max(k.get(name, 0), cum)
                        sem_hist.setdefault(name, []).append((cum, dict(k)))
            new_instructions.append(inst)
        if len(new_instructions) != len(b.instructions):
            b.instructions = new_instructions


def kernel(llr, max_iters):
    llr = np.ascontiguousarray(np.asarray(llr), dtype=np.float32)
    iters = int(np.asarray(max_iters))
    B = llr.shape[0]
    if iters <= 0:
        return llr.reshape(B, 1, 7).copy()

    from concourse.bass_utils import run_bass_kernel_spmd

    Bc = B // NCORES
    key = (Bc, iters)
    if key not in _CACHE:
        _CACHE[key] = _build(Bc, iters)
    nc = _CACHE[key]

    flat = llr.reshape(B, 7)
    in_maps = [{"llr": flat[i * Bc:(i + 1) * Bc]} for i in range(NCORES)]
    res = run_bass_kernel_spmd(nc, in_maps, core_ids=list(range(NCORES)))
    out = np.concatenate([np.asarray(r["out"]) for r in res.results], axis=0)
    return out.reshape(B, 1, 7)


# revision 46
# speedup vs baseline: 2.6790x; 1.0273x over previous
"""LDPC belief-propagation kernel for Trainium2 (8 NeuronCores, data-parallel).

Math (per batch row, H fixed [3,7], 12 edges):
  t_e   = tanh(m_e / 2)                       (signed!)
  u_e   = prod_{e' in check c, e' != e} t_e'  (signed leave-one-out product)
  c2v_e = 2 atanh(u_e) = ln(1+u) - ln(1-u)    (signed, sign handled for free)
  new_llr_v = llr_v + sum_{c contains v} c2v_{c,v}
  m'_e  = new_llr_v - c2v_e
Only Tanh/Ln tables; the sign pipeline of the classic phi/phi formulation
disappears because the tanh products carry signs natively.

Edge layout is role-major per 12-slot group: [s0 s1 s2 | a0 a1 a2 | b0 b1 b2 |
d0 d1 d2] where s_c is the check's degree-1-variable edge (v0,v1,v3 - their
messages never change), d_c is v6's edge in check c, and (a, b) =
((v2c0, v2c1, v4c2), (v4c0, v5c1, v5c2)).  This makes every structural op a
single strided instruction:
  Q[k]   = T[k] * T[k+6]         (k=0..5: pair products (s*b, a*d) per check)
  U[3..8]  = T[(9,10,11,0,1,2)] * Q[0..5]   (loo for roles a, b)
  U[9..11] = T[3..5] * Q[0..2]              (loo for role d)
  M'[deg2 six edges] = LB6 + CV[partner]    (partner = pair-swap view)
  M'[d]  = (c2v_d-sum + llr_6) - CV[d]      (v6 leave-one-out via total sum)
Batch is split into 2 chunks so ACT (Tanh/Ln/Ln), DVE (products, M-updates)
and Pool (CV subtract, v6 sums) pipeline across chunks.
"""

import numpy as np

_CACHE = {}

NCORES = 8
P = 128      # partitions
CHUNKS = 3   # batch sub-chunks per core (pipeline depth)

# guard so ln(1 -+ 0.99999988*u) stays finite (>= ~1.2e-7) even at u = -+1
LNSCALE = 0.99999988

# (start, half-period, partA->partB gap) ns hints for the list scheduler
_SCHED = (0, 0, 0)

# schedule hints deferring chunk setup copies (ns)
_CPY = (0, 9800, 12300)

# manual chunk widths (must sum to Bc//P//1); None = near-even
_WS = (84, 88, 84)


def _build(Bc, iters):
    import contextlib

    import concourse.bass as bass
    import concourse.tile as tile
    from concourse import mybir
    from concourse.alu_op_type import AluOpType as Op

    F = mybir.ActivationFunctionType
    Wtot = Bc // P
    if _WS is not None and sum(_WS) == Wtot:
        Ws = list(_WS)
    else:
        base, rem = divmod(Wtot, CHUNKS)
        Ws = [base + (1 if i == 1 else 0) for i in range(CHUNKS)] \
            if rem == 1 else \
            [base + (1 if i >= CHUNKS - rem else 0) for i in range(CHUNKS)]
    f32 = mybir.dt.float32

    f16 = mybir.dt.float16
    nc = bass.Bass("TRN2", target_bir_lowering=False, debug=False,
                   num_devices=1)
    llr_d = nc.dram_tensor("llr", [Bc, 7], f32, kind="ExternalInput")
    out_d = nc.dram_tensor("out", [Bc, 7], f32, kind="ExternalOutput")

    def sub(t, off, dims):
        a = t[:] if callable(getattr(t, "__getitem__", None)) else t
        return bass.AP(tensor=a.tensor, offset=a.offset + off,
                       ap=[list(a.ap[0])] + [list(d) for d in dims])

    with tile.TileContext(nc) as tc:
        ctx = contextlib.ExitStack()
        with ctx:
            keep = ctx.enter_context(tc.tile_pool(name="keep", bufs=1))
            work = ctx.enter_context(tc.tile_pool(name="work", bufs=2))

            def K(name, c, k, dt=f32):
                return keep.tile([P, Ws[c] * k], dt, tag=name, name=name)

            # per-chunk persistent state
            LLs = [K(f"LL{c}", c, 7) for c in range(CHUNKS)]    # llr, natural v order
            LBs = [K(f"LB{c}", c, 6, f16) for c in range(CHUNKS)]   # llr bcast, deg2 edges
            L6s = [K(f"L6{c}", c, 3, f16) for c in range(CHUNKS)]   # llr6 bcast, v6 edges
            Ts  = [K(f"T{c}", c, 12) for c in range(CHUNKS)]    # tanh(m/2) per edge
            Ms  = [K(f"M{c}", c, 9, f16) for c in range(CHUNKS)]    # dyn messages
            NLs = [K(f"NL{c}", c, 7) for c in range(CHUNKS)]    # output llr

            act = nc.scalar.activation
            vec = nc.vector
            gps = nc.gpsimd

            def dram_view(t, c, w0, nw):
                # [P, nw*7] window of chunk c: rows base_c + p*Ws[c] + w
                a = t.ap()
                off = (P * sum(Ws[:c]) + w0) * 7
                return bass.AP(tensor=a.tensor, offset=a.offset + off,
                               ap=[[Ws[c] * 7, P], [1, nw * 7]])

            for c in range(CHUNKS):
                eng = nc.sync if c == 0 else nc.gpsimd
                eng.dma_start(out=LLs[c][:], in_=dram_view(llr_d, c, 0, Ws[c]))

            cur = {"W": Ws[0]}

            def v7(t, off, *dims):
                return sub(t, off, [[7, cur["W"]]] + [list(d) for d in dims])

            def v12(t, off, *dims):
                return sub(t, off, [[12, cur["W"]]] + [list(d) for d in dims])

            def v9(t, off, *dims):
                return sub(t, off, [[9, cur["W"]]] + [list(d) for d in dims])

            def v6(t, off, *dims):
                return sub(t, off, [[6, cur["W"]]] + [list(d) for d in dims])

            def v3(t, off, *dims):
                return sub(t, off, [[3, cur["W"]]] + [list(d) for d in dims])

            state = [{} for _ in range(CHUNKS)]

            def partA(c, it):
                """tanh + products: T, Q, U."""
                LL, LB, T, M = LLs[c], LBs[c], Ts[c], Ms[c]
                last = (it == iters - 1)
                W = Ws[c]
                cur["W"] = W
                Q = work.tile([P, W * 6], f32, tag=f"Q{c}", name="Q")
                U = work.tile([P, W * 12], f32, tag=f"U{c}", name="U")
                state[c] = {"Q": Q, "U": U}

                if it == 0:
                    # t = tanh(llr/2) once; iteration-0 products read TL
                    # directly so the T scatter stays off the critical path
                    TL = work.tile([P, W * 7], f32, tag=f"TL{c}", name="TL")
                    act(TL[:], LL[:], F.Tanh, scale=0.5)
                    vec.tensor_tensor(v6(Q, 0, [1, 2]), v7(TL, 0, [1, 2]),
                                      v7(TL, 4, [1, 2]), Op.mult)
                    vec.tensor_tensor(v6(Q, 2, [1, 1]), v7(TL, 3, [1, 1]),
                                      v7(TL, 5, [1, 1]), Op.mult)
                    gps.tensor_tensor(v6(Q, 3, [1, 2]), v7(TL, 2, [0, 2]),
                                      v7(TL, 6, [0, 2]), Op.mult)
                    gps.tensor_tensor(v6(Q, 5, [1, 1]), v7(TL, 4, [1, 1]),
                                      v7(TL, 6, [1, 1]), Op.mult)
                    vec.tensor_tensor(v12(U, 3, [1, 3]), v7(TL, 6, [0, 3]),
                                      v6(Q, 0, [1, 3]), Op.mult)
                    vec.tensor_tensor(v12(U, 6, [1, 2]), v7(TL, 0, [1, 2]),
                                      v6(Q, 3, [1, 2]), Op.mult)
                    vec.tensor_tensor(v12(U, 8, [1, 1]), v7(TL, 3, [1, 1]),
                                      v6(Q, 5, [1, 1]), Op.mult)
                    gps.tensor_tensor(v12(U, 9, [1, 2]), v7(TL, 2, [0, 2]),
                                      v6(Q, 0, [1, 2]), Op.mult)
                    gps.tensor_tensor(v12(U, 11, [1, 1]), v7(TL, 4, [1, 1]),
                                      v6(Q, 2, [1, 1]), Op.mult)
                    if last:  # iters == 1
                        vec.tensor_tensor(v12(U, 0, [1, 2]), v7(TL, 4, [1, 2]),
                                          v6(Q, 3, [1, 2]), Op.mult)
                        vec.tensor_tensor(v12(U, 2, [1, 1]), v7(TL, 5, [1, 1]),
                                          v6(Q, 5, [1, 1]), Op.mult)
                    # scatter t to role-major slots for later iterations,
                    # off the critical path (only statics strictly needed
                    # before iteration 1's products)
                    vec.tensor_copy(v12(T, 0, [1, 2]), v7(TL, 0, [1, 2]))
                    vec.tensor_copy(v12(T, 2, [1, 1]), v7(TL, 3, [1, 1]))
                    with tc.tile_wait_until(_CPY[c] / 1e6, enable=_CPY[c] > 0):
                        gps.tensor_copy(v6(LB, 0, [1, 2]), v7(LL, 2, [0, 2]))
                        gps.tensor_copy(v6(LB, 2, [1, 4]),
                                        v7(LL, 4, [1, 2], [0, 2]))
                        gps.tensor_copy(v3(L6s[c], 0, [1, 3]),
                                        v7(LL, 6, [0, 3]))
                else:
                    act(v12(T, 3, [1, 9]), M[:], F.Tanh, scale=0.5)
                    # pair products and signed leave-one-out products
                    vec.tensor_tensor(Q[:], v12(T, 0, [1, 6]),
                                      v12(T, 6, [1, 6]), Op.mult)
                    vec.tensor_tensor(v12(U, 3, [1, 6]),
                                      v12(T, 9, [-9, 2], [1, 3]),
                                      v6(Q, 0, [1, 6]), Op.mult)
                    gps.tensor_tensor(v12(U, 9, [1, 3]), v12(T, 3, [1, 3]),
                                      v6(Q, 0, [1, 3]), Op.mult)
                    if last:
                        vec.tensor_tensor(v12(U, 0, [1, 3]), v12(T, 6, [1, 3]),
                                          v6(Q, 3, [1, 3]), Op.mult)

            def partB(c, it):
                """c2v + message/new-llr update."""
                LL, LB, M, NL = LLs[c], LBs[c], Ms[c], NLs[c]
                last = (it == iters - 1)
                W = Ws[c]
                cur["W"] = W
                U = state[c]["U"]
                LP = work.tile([P, W * 12], f16, tag=f"LP{c}", name="LP")
                LM = work.tile([P, W * 12], f16, tag=f"LM{c}", name="LM")
                CV = work.tile([P, W * 12], f16, tag=f"CV{c}", name="CV")

                off, n = (0, 9) if last else (3, 6)
                # c2v = ln(1+u) - ln(1-u), guarded away from ln(0)
                act(v12(LP, off, [1, n + 3]), v12(U, off, [1, n + 3]), F.Ln,
                    bias=1.0, scale=LNSCALE)
                act(v12(LM, off, [1, n + 3]), v12(U, off, [1, n + 3]), F.Ln,
                    bias=1.0, scale=-LNSCALE)
                vec.tensor_tensor(v12(CV, off, [1, n + 3]),
                                  v12(LP, off, [1, n + 3]),
                                  v12(LM, off, [1, n + 3]), Op.subtract)

                # v6 leave-one-out sums of the d-role c2vs, depth 2:
                # X[0]=c10+c11  X[1]=c9+c11  X[2]=c9+c10
                X = work.tile([P, W * 3], f16, tag=f"X{c}", name="X")
                vec.tensor_tensor(v3(X, 0, [1, 2]), v12(CV, 10, [-1, 2]),
                                  v12(CV, 11, [0, 2]), Op.add)
                gps.tensor_tensor(v3(X, 2, [1, 1]), v12(CV, 9, [1, 1]),
                                  v12(CV, 10, [1, 1]), Op.add)

                if not last:
                    # m' for the six deg-2 edges: llr + partner c2v
                    vec.tensor_tensor(v9(M, 0, [1, 6]), v6(LB, 0, [1, 6]),
                                      v12(CV, 4, [2, 3], [-1, 2]), Op.add)
                    # m' for v6 edges: llr6 + sum of the other two c2v_d
                    vec.tensor_tensor(v9(M, 6, [1, 3]), v3(L6s[c], 0, [1, 3]),
                                      v3(X, 0, [1, 3]), Op.add)
                else:
                    # new_llr in natural variable order
                    SP = work.tile([P, W * 3], f32, tag=f"SP{c}", name="SP")
                    gps.tensor_tensor(v7(NL, 0, [1, 2]), v7(LL, 0, [1, 2]),
                                      v12(CV, 0, [1, 2]), Op.add)
                    gps.tensor_tensor(v7(NL, 3, [1, 1]), v7(LL, 3, [1, 1]),
                                      v12(CV, 2, [1, 1]), Op.add)
                    vec.tensor_tensor(SP[:], v12(CV, 3, [2, 3]),
                                      v12(CV, 4, [2, 3]), Op.add)
                    vec.tensor_tensor(v7(NL, 2, [1, 1]), v7(LL, 2, [1, 1]),
                                      v3(SP, 0, [1, 1]), Op.add)
                    vec.tensor_tensor(v7(NL, 4, [1, 2]), v7(LL, 4, [1, 2]),
                                      v3(SP, 1, [1, 2]), Op.add)
                    S1 = work.tile([P, W], f32, tag=f"S1{c}", name="S1")
                    vec.tensor_tensor(S1[:], v3(X, 2, [1, 1]),
                                      v12(CV, 11, [1, 1]), Op.add)
                    vec.tensor_tensor(v7(NL, 6, [1, 1]), S1[:],
                                      v7(LL, 6, [1, 1]), Op.add)
                    wl = W // 2
                    wh = W - wl
                    lo = bass.AP(tensor=NL[:].tensor, offset=NL[:].offset,
                                 ap=[list(NL[:].ap[0])] + [[7, wl], [1, 7]])
                    hi = bass.AP(tensor=NL[:].tensor,
                                 offset=NL[:].offset + wl * 7,
                                 ap=[list(NL[:].ap[0])] + [[7, wh], [1, 7]])
                    e0, e1 = ((nc.sync, nc.sync) if c < CHUNKS - 1
                              else (nc.sync, nc.gpsimd))
                    e0.dma_start(out=dram_view(out_d, c, 0, wl), in_=lo)
                    e1.dma_start(out=dram_view(out_d, c, wl, wh), in_=hi)

            # software-pipelined schedule: chunk 1 runs half an iteration
            # behind chunk 0 so each chunk's ACT phase (Tanh / Ln Ln) overlaps
            # the other chunk's vector phase (products / updates).  The
            # wait-until timestamps steer the Tile list scheduler into that
            # stagger; they are lower bounds only, data deps still rule.
            S0, HALF, GAP = _SCHED
            for it in range(iters):
                for c in range(CHUNKS):
                    tA = S0 + (CHUNKS * it + c) * HALF
                    with tc.tile_wait_until(tA / 1e6, enable=tA > 0):
                        partA(c, it)
                    with tc.tile_wait_until((tA + GAP) / 1e6):
                        partB(c, it)

    _strip_syncs(nc)
    return nc


def _strip_syncs(nc):
    """walrus on this stack supports a single sync-wait slot per instruction.
    Reduce each instruction's wait list via a vector-clock pass: walking the
    scheduled program order, every engine accumulates knowledge of semaphore
    values - from its own queue position, from waits it has already performed,
    and transitively from the producer's knowledge snapshot at the awaited
    update.  A wait already implied by that knowledge is dropped.  Kernel-tail
    drains keep only their DMA wait (the per-engine drain + EVSEM butterfly
    that follows enforces engine completion)."""
    import bass_rust

    eng_sem = {"EngineType.DVE": "DVE_", "EngineType.Pool": "Pool_",
               "EngineType.Activation": "Activation_", "EngineType.PE": "PE_",
               "EngineType.SP": "SP_"}
    know = {e: {} for e in eng_sem}          # engine -> {sem: value}
    sem_hist = {}                            # sem -> list of (cum_value, snapshot)
    sem_cum = {}                             # sem -> cumulative inc so far

    # Sems that are ever decremented (barrier gather sems) are not monotone;
    # leave their waits untouched and keep them out of the knowledge model.
    nonmono = set()
    for b in nc.m.functions[0].blocks:
        for inst in b.instructions:
            si = inst.sync_info
            if si is not None:
                for u in si.on_update:
                    if u.update_mode != "sem-inc":
                        nonmono.add(u.ant_name)

    def implied(k, sem, val):
        return k.get(sem, 0) >= val

    def learn(k, sem, val):
        if k.get(sem, 0) < val:
            k[sem] = val
        # transitively absorb the producer's snapshot at this update
        hist = sem_hist.get(sem)
        if hist:
            import bisect
            i = bisect.bisect_left([h[0] for h in hist], val)
            if i < len(hist):
                for s2, v2 in hist[i][1].items():
                    if k.get(s2, 0) < v2:
                        k[s2] = v2

    from concourse import mybir

    for b in nc.m.functions[0].blocks:
        new_instructions = []
        for inst in b.instructions:
            si = inst.sync_info
            eng = str(inst.engine)
            k = know.setdefault(eng, {})
            if si is not None:
                waits = list(si.on_wait)
                if type(inst).__name__ == "InstDrain" and len(waits) > 1:
                    dma = [w for w in waits if "DMA" in w.ant_name]
                    keep_w = dma[-1:] if dma else waits[:1]
                    for w in waits:
                        learn(k, w.ant_name, w.wait_value)
                else:
                    merged = {}
                    for w in waits:
                        if w.ant_name in nonmono:
                            merged[id(w)] = w
                        elif w.ant_name not in merged or \
                                merged[w.ant_name].wait_value < w.wait_value:
                            merged[w.ant_name] = w
                    keep_w = []
                    for w in merged.values():
                        if w.ant_name in nonmono:
                            keep_w.append(w)
                            continue
                        if not implied(k, w.ant_name, w.wait_value):
                            keep_w.append(w)
                        learn(k, w.ant_name, w.wait_value)
                    # walrus has one wait slot per instruction: hoist extra
                    # waits onto injected no-ops on the same engine
                    while len(keep_w) > 1:
                        w = keep_w.pop(0)
                        nop = mybir.InstNoOp(
                            name=f"{inst.name}_w{len(keep_w)}",
                            engine=inst.engine, ins=[], outs=[],
                            sync_info=bass_rust.SyncInfo(
                                on_wait=[w], on_update=[]))
                        new_instructions.append(nop)
                if len(keep_w) != len(waits):
                    inst.sync_info = bass_rust.SyncInfo(
                        on_wait=keep_w, on_update=list(si.on_update))
                    si = inst.sync_info
                for u in si.on_update:
                    if u.update_mode == "sem-inc" and u.ant_name not in nonmono:
                        name = u.ant_name
                        cum = sem_cum.get(name, 0) + u.update_value
                        sem_cum[name] = cum
                        # own-engine sems are implicitly ordered for later
                        # instructions on the same queue
                        pref = eng_sem.get(eng)
                        if pref and name.startswith(pref):
                            k[name] = max(k.get(name, 0), cum)
                        sem_hist.setdefault(name, []).append((cum, dict(k)))
            new_instructions.append(inst)
        if len(new_instructions) != len(b.instructions):
            b.instructions = new_instructions


def kernel(llr, max_iters):
    llr = np.ascontiguousarray(np.asarray(llr), dtype=np.float32)
    iters = int(np.asarray(max_iters))
    B = llr.shape[0]
    if iters <= 0:
        return llr.reshape(B, 1, 7).copy()

    from concourse.bass_utils import run_bass_kernel_spmd

    Bc = B // NCORES
    key = (Bc, iters)
    if key not in _CACHE:
        _CACHE[key] = _build(Bc, iters)
    nc = _CACHE[key]

    flat = llr.reshape(B, 7)
    in_maps = [{"llr": flat[i * Bc:(i + 1) * Bc]} for i in range(NCORES)]
    res = run_bass_kernel_spmd(nc, in_maps, core_ids=list(range(NCORES)))
    out = np.concatenate([np.asarray(r["out"]) for r in res.results], axis=0)
    return out.reshape(B, 1, 7)
